# revision 12
# baseline (speedup 1.0000x reference)
"""BranchedAttention Trainium2 kernel (8-core head-parallel SPMD).

Strategy (head parallel per sharding hint):
  - core n owns head n: projections, attention, folded output projection.
  - w_o @ fc_w are back-to-back linear maps (selu comes after fc_w), so they
    are folded on device into W2[n] = softmax(k_gate)[n] * (w_o[n] @ fc_w).
  - scores computed twice (natural [q,s] for softmax+output, transposed [s,q]
    for the attn@v matmul) to avoid on-device fp32 transposes of attention.
    attn@v consumes unnormalized exp; the softmax denominator is folded in
    after the V matmul (linearity).
  - head-sum via ReduceScatter per batch; residual+layernorm on the shard.

Self-contained: hardcodes all shapes from the problem spec.
"""

import numpy as np

B = 4
T = 1024          # q_len == seq_len
H = 1024          # hidden
NH = 8            # heads
DK = 128          # per-head dim
E = 2 * H         # fc input dim
N_CORES = 8
LN_EPS = 1e-6
INV_SQRT_DK = 1.0 / float(np.sqrt(DK))
SELU_SCALE = 1.0507009873554805
SELU_ALPHA = 1.6732632423543772

_CACHE = {}


def _build():
    import concourse.bass as bass
    import concourse.mybir as mybir
    import concourse.tile as tile
    from concourse import bacc
    from concourse.masks import make_identity
    from concourse.tile_rust import add_dep_helper

    f32 = mybir.dt.float32
    f32r = mybir.dt.float32r
    bf16 = mybir.dt.bfloat16
    u32 = mybir.dt.uint32
    AF = mybir.ActivationFunctionType
    ALU = mybir.AluOpType

    nc = bacc.Bacc("TRN2", target_bir_lowering=False, debug=False,
                   num_devices=N_CORES)

    # ---- I/O ----
    qT_d = nc.dram_tensor("qT", [H, B * T], f32r, kind="ExternalInput")
    kT_d = nc.dram_tensor("kT", [H, B * T], f32r, kind="ExternalInput")
    vT_d = nc.dram_tensor("vT", [H, B * T], f32r, kind="ExternalInput")
    wq_d = nc.dram_tensor("wq", [H, DK], f32r, kind="ExternalInput")
    wk_d = nc.dram_tensor("wk", [H, DK], f32r, kind="ExternalInput")
    wv_d = nc.dram_tensor("wv", [H, DK], f32r, kind="ExternalInput")
    wo_d = nc.dram_tensor("wo", [DK, E], f32r, kind="ExternalInput")
    fcw_d = nc.dram_tensor("fcw", [E, H], bf16, kind="ExternalInput")
    fcb_d = nc.dram_tensor("fcb", [1, H], bf16, kind="ExternalInput")
    ag_d = nc.dram_tensor("ag", [1, NH], f32, kind="ExternalInput")
    kg_d = nc.dram_tensor("kg", [1, NH], f32, kind="ExternalInput")
    sel_d = nc.dram_tensor("sel", [1, NH], f32, kind="ExternalInput")
    lng_d = nc.dram_tensor("lng", [1, H], f32, kind="ExternalInput")
    lnb_d = nc.dram_tensor("lnb", [1, H], f32, kind="ExternalInput")
    qres_d = nc.dram_tensor("qres", [B, 128, H], f32, kind="ExternalInput")

    attn_d = nc.dram_tensor("attn_out", [B, T, T], f32, kind="ExternalOutput")
    y_d = nc.dram_tensor("y_out", [B, 128, H], f32, kind="ExternalOutput")

    def rsqrt_newton(pool, varr, name):
        """1/sqrt(varr) on [128,1] via quake initial guess + 3 Newton steps."""
        t0 = pool.tile([128, 1], u32, name=f"{name}_t0")
        nc.vector.tensor_scalar(t0[:], varr[:].bitcast(u32), 1, None,
                                op0=ALU.logical_shift_right)
        y0 = pool.tile([128, 1], f32, name=f"{name}_y0")
        nc.vector.tensor_tensor(y0[:].bitcast(u32), magic[:], t0[:],
                                op=ALU.subtract)
        ah = pool.tile([128, 1], f32, name=f"{name}_ah")
        nc.vector.tensor_scalar(ah[:], varr[:], 0.5, None, op0=ALU.mult)
        ycur = y0
        for it in range(3):
            sq = pool.tile([128, 1], f32, name=f"{name}_sq{it}")
            nc.vector.tensor_tensor(sq[:], ycur[:], ycur[:], op=ALU.mult)
            u = pool.tile([128, 1], f32, name=f"{name}_u{it}")
            nc.vector.tensor_tensor(u[:], sq[:], ah[:], op=ALU.mult)
            w2c = pool.tile([128, 1], f32, name=f"{name}_w2{it}")
            nc.vector.tensor_scalar(w2c[:], u[:], -1.0, 1.5,
                                    op0=ALU.mult, op1=ALU.add)
            yn = pool.tile([128, 1], f32, name=f"{name}_yn{it}")
            nc.vector.tensor_tensor(yn[:], ycur[:], w2c[:], op=ALU.mult)
            ycur = yn
        return ycur

    with tile.TileContext(nc) as tc:
        with (
            tc.tile_pool(name="const", bufs=1) as cpool,
            tc.tile_pool(name="wpool", bufs=1) as wpool,
        ):
            # ---- constants ----
            ident = cpool.tile([128, 128], f32)
            make_identity(nc, ident[:])
            magic = cpool.tile([128, 1], u32)
            nc.gpsimd.memset(magic[:], 0x5F3759DF)

            # ---- gate softmaxes; select this head's entries ----
            ag_t = cpool.tile([1, NH], f32)
            kg_t = cpool.tile([1, NH], f32)
            sel_t = cpool.tile([1, NH], f32)
            nc.sync.dma_start(ag_t[:], ag_d[:])
            nc.sync.dma_start(kg_t[:], kg_d[:])
            nc.sync.dma_start(sel_t[:], sel_d[:])

            def gate_scalar(gt, name):
                ge = cpool.tile([1, NH], f32, name=f"{name}_ge")
                gs = cpool.tile([1, 1], f32, name=f"{name}_gs")
                nc.scalar.activation(ge[:], gt[:], AF.Exp, accum_out=gs[:])
                gr = cpool.tile([1, 1], f32, name=f"{name}_gr")
                nc.vector.reciprocal(gr[:], gs[:])
                picked = cpool.tile([1, NH], f32, name=f"{name}_picked")
                nc.vector.tensor_tensor(picked[:], ge[:], sel_t[:], op=ALU.mult)
                psum_ = cpool.tile([1, 1], f32, name=f"{name}_psum")
                nc.vector.reduce_sum(psum_[:], picked[:],
                                     axis=mybir.AxisListType.X)
                out = cpool.tile([1, 1], f32, name=f"{name}_sm")
                nc.vector.tensor_tensor(out[:], psum_[:], gr[:], op=ALU.mult)
                return out

            sma = gate_scalar(ag_t, "a")     # softmax(a_gate)[n]
            smk = gate_scalar(kg_t, "k")     # softmax(k_gate)[n]
            sma_bc = cpool.tile([128, 1], f32)
            smk_bc = cpool.tile([128, 1], f32)
            nc.gpsimd.partition_broadcast(sma_bc[:], sma[:])
            nc.gpsimd.partition_broadcast(smk_bc[:], smk[:])
            c1 = cpool.tile([128, 1], f32)   # selu_scale * sm_a
            nc.vector.tensor_scalar(c1[:], sma_bc[:], SELU_SCALE, None,
                                    op0=ALU.mult)
            c2 = cpool.tile([128, 1], f32)   # selu_scale * alpha * sm_a
            nc.vector.tensor_scalar(c2[:], sma_bc[:], SELU_SCALE * SELU_ALPHA,
                                    None, op0=ALU.mult)
            lnc2 = cpool.tile([128, 1], f32)
            nc.scalar.activation(lnc2[:], c2[:], AF.Ln)

            # ---- LN gamma/beta broadcast ----
            lng_row = cpool.tile([1, H], f32)
            lnb_row = cpool.tile([1, H], f32)
            nc.sync.dma_start(lng_row[:], lng_d[:])
            nc.sync.dma_start(lnb_row[:], lnb_d[:])
            lng_bc = cpool.tile([128, H], f32)
            lnb_bc = cpool.tile([128, H], f32)
            nc.gpsimd.partition_broadcast(lng_bc[:], lng_row[:])
            nc.gpsimd.partition_broadcast(lnb_bc[:], lnb_row[:])

            # ---- per-head projection weights ----
            wq_sb = wpool.tile([128, H], f32r)   # col block h: wq[h*128:+128,:]
            wk_sb = wpool.tile([128, H], f32r)
            wv_sb = wpool.tile([128, H], f32r)
            for h in range(8):
                nc.sync.dma_start(wq_sb[:, h * 128:(h + 1) * 128],
                                  wq_d[h * 128:(h + 1) * 128, :])
                nc.sync.dma_start(wk_sb[:, h * 128:(h + 1) * 128],
                                  wk_d[h * 128:(h + 1) * 128, :])
                nc.sync.dma_start(wv_sb[:, h * 128:(h + 1) * 128],
                                  wv_d[h * 128:(h + 1) * 128, :])

            fcb_sb = cpool.tile([1, H], bf16)
            nc.sync.dma_start(fcb_sb[:], fcb_d[:])

            # ---- fold W2 = smk * (w_o @ fc_w)  [DK, H] ----
            W2_sb = wpool.tile([128, H], bf16)
            with (
                tc.tile_pool(name="stage", bufs=1) as stage,
                tc.tile_pool(name="stage_ps", bufs=2, space="PSUM") as stage_ps,
            ):
                wo_sb = stage.tile([128, E], f32r)
                nc.sync.dma_start(wo_sb[:], wo_d[:])
                # scale by softmax(k_gate)[n]
                nc.vector.tensor_scalar(wo_sb[:].bitcast(f32),
                                        wo_sb[:].bitcast(f32), smk_bc[:],
                                        None, op0=ALU.mult)
                # transpose wo -> woT [E, DK] (16 col blocks of [128,128])
                woT_sb = stage.tile([128, 16 * 128], bf16)
                for g in range(4):
                    tr_ps = stage_ps.tile([128, 512], f32, name="tr_ps")
                    for j in range(4):
                        e = g * 4 + j
                        nc.tensor.transpose(
                            tr_ps[:, j * 128:(j + 1) * 128],
                            wo_sb[:, e * 128:(e + 1) * 128].bitcast(f32),
                            ident[:])
                    nc.vector.tensor_copy(
                        woT_sb[:, g * 512:(g + 1) * 512], tr_ps[:])
                # fcw chunks + accumulate W2
                w2_halves = []
                for half in range(2):
                    w2_ps_h = stage_ps.tile([128, 512], f32,
                                            name=f"w2_ps{half}", bufs=1)
                    w2_halves.append(w2_ps_h)
                for e in range(16):
                    fcw_t = stage.tile([128, H], bf16, name="fcw_t", bufs=3)
                    nc.sync.dma_start(fcw_t[:],
                                      fcw_d[e * 128:(e + 1) * 128, :])
                    for half in range(2):
                        nc.tensor.matmul(
                            w2_halves[half][:],
                            woT_sb[:, e * 128:(e + 1) * 128],
                            fcw_t[:, half * 512:(half + 1) * 512],
                            start=(e == 0), stop=(e == 15))
                for half in range(2):
                    nc.vector.tensor_copy(
                        W2_sb[:, half * 512:(half + 1) * 512],
                        w2_halves[half][:])

            # ---- main pools ----
            with (
                tc.tile_pool(name="stream", bufs=3) as stream,
                tc.tile_pool(name="proj", bufs=2) as proj,
                tc.tile_pool(name="apool", bufs=2) as apool,
                tc.tile_pool(name="fpool", bufs=2) as fpool,
                tc.tile_pool(name="lnpool", bufs=2) as lnpool,
                tc.tile_pool(name="mps", bufs=1, space="PSUM") as mps,
                tc.tile_pool(name="dpool", bufs=2, space="DRAM") as dpool,
            ):
                cc_outs = []
                for b in range(B):
                    c0 = b * T  # column offset of this batch in [H, B*T]

                    # ---------- phase A: projections ----------
                    qh_ps = mps.tile([128, T], f32, name="qh_ps", tag="acc1")
                    kh_ps = mps.tile([128, T], f32, name="kh_ps", tag="acc2")
                    for h in range(8):
                        qch = stream.tile([128, T], f32r, name="qch")
                        nc.sync.dma_start(
                            qch[:], qT_d[h * 128:(h + 1) * 128, c0:c0 + T])
                        for hf in range(2):
                            nc.tensor.matmul(
                                qh_ps[:, hf * 512:(hf + 1) * 512],
                                wq_sb[:, h * 128:(h + 1) * 128],
                                qch[:, hf * 512:(hf + 1) * 512],
                                start=(h == 0), stop=(h == 7))
                        kch = stream.tile([128, T], f32r, name="kch")
                        nc.sync.dma_start(
                            kch[:], kT_d[h * 128:(h + 1) * 128, c0:c0 + T])
                        for hf in range(2):
                            nc.tensor.matmul(
                                kh_ps[:, hf * 512:(hf + 1) * 512],
                                wk_sb[:, h * 128:(h + 1) * 128],
                                kch[:, hf * 512:(hf + 1) * 512],
                                start=(h == 0), stop=(h == 7))
                    qhT_sb = proj.tile([128, T], f32r, name="qhT_sb")
                    khT_sb = proj.tile([128, T], f32r, name="khT_sb")
                    nc.vector.tensor_copy(qhT_sb[:], qh_ps[:])
                    nc.scalar.copy(khT_sb[:], kh_ps[:])
                    vh_ps = mps.tile([128, T], f32, name="vh_ps", tag="acc1")
                    for h in range(8):
                        vch = stream.tile([128, T], f32r, name="vch")
                        nc.sync.dma_start(
                            vch[:], vT_d[h * 128:(h + 1) * 128, c0:c0 + T])
                        for hf in range(2):
                            nc.tensor.matmul(
                                vh_ps[:, hf * 512:(hf + 1) * 512],
                                wv_sb[:, h * 128:(h + 1) * 128],
                                vch[:, hf * 512:(hf + 1) * 512],
                                start=(h == 0), stop=(h == 7))
                    # vhT [d, s] -> transpose to vh natural [s, d] col blocks
                    vhT_sb = proj.tile([128, T], f32, name="vhT_sb")
                    nc.vector.tensor_copy(vhT_sb[:], vh_ps[:])
                    vh_sb = proj.tile([128, T], bf16, name="vh_sb")
                    for g in range(2):
                        tr_ps = mps.tile([128, 512], f32, name="tr_ps",
                                         tag=f"rot{g}")
                        for j in range(4):
                            st = g * 4 + j
                            nc.tensor.transpose(
                                tr_ps[:, j * 128:(j + 1) * 128],
                                vhT_sb[:, st * 128:(st + 1) * 128],
                                ident[:])
                        nc.vector.tensor_copy(
                            vh_sb[:, g * 512:(g + 1) * 512], tr_ps[:])

                    # ---------- phase B: scores + softmax (natural) ----------
                    sumsT_sb = apool.tile([1, T], bf16, name="sumsT_sb")
                    recips = []
                    for qt in range(8):
                        sc_ps = mps.tile([128, T], f32, name="sc_ps",
                                         tag=f"rot{qt % 2}")
                        for hf in range(2):
                            nc.tensor.matmul(
                                sc_ps[:, hf * 512:(hf + 1) * 512],
                                qhT_sb[:, qt * 128:(qt + 1) * 128],
                                khT_sb[:, hf * 512:(hf + 1) * 512],
                                start=True, stop=True)
                        exp_sb = apool.tile([128, T], f32, name="exp_sb")
                        sums = apool.tile([128, 1], f32, name="sums")
                        nc.scalar.activation(exp_sb[:], sc_ps[:], AF.Exp,
                                             scale=INV_SQRT_DK,
                                             accum_out=sums[:])
                        recip = apool.tile([128, 1], f32, name="recip",
                                           bufs=12)
                        recips.append(recip)
                        nc.vector.reciprocal(recip[:], sums[:])
                        nc.vector.tensor_scalar(exp_sb[:], exp_sb[:],
                                                recip[:], None, op0=ALU.mult)
                        nc.sync.dma_start(
                            attn_d[b, qt * 128:(qt + 1) * 128, :], exp_sb[:])
                        # sums -> transposed [1,128] into sumsT_sb (bias row)
                        rtp = mps.tile([1, 128], f32, name="rtp", tag="acc2")
                        nc.tensor.transpose(rtp[:], sums[:], ident[:])
                        nc.vector.tensor_copy(
                            sumsT_sb[0:1, qt * 128:(qt + 1) * 128], rtp[:])

                    # ---------- phase C+D: scoresT + exp, attn @ v ----------
                    outT_ps = mps.tile([128, T], f32, name="outT_ps",
                                       tag="acc1")
                    for st in range(8):
                        scT_ps = mps.tile([128, T], f32, name="scT_ps",
                                          tag=f"rot{st % 2}")
                        for hf in range(2):
                            nc.tensor.matmul(
                                scT_ps[:, hf * 512:(hf + 1) * 512],
                                khT_sb[:, st * 128:(st + 1) * 128],
                                qhT_sb[:, hf * 512:(hf + 1) * 512],
                                start=True, stop=True)
                        expT = apool.tile([128, T], bf16, name="expT", bufs=2)
                        nc.scalar.activation(expT[:], scT_ps[:],
                                             AF.Exp, scale=INV_SQRT_DK)
                        for hf in range(2):
                            nc.tensor.matmul(
                                outT_ps[:, hf * 512:(hf + 1) * 512],
                                vh_sb[:, st * 128:(st + 1) * 128],
                                expT[:, hf * 512:(hf + 1) * 512],
                                start=(st == 0), stop=(st == 7))
                    outT_sb = proj.tile([128, T], bf16, name="outT_sb")
                    nc.vector.tensor_copy(outT_sb[:], outT_ps[:])

                    # ---------- phase E: f = outT.T @ W2 + fcb; selu; gate ---
                    cc_in = dpool.tile([T, H], bf16, name="cc_in")
                    for tt in range(8):
                        f_ps = mps.tile([128, H], f32, name="f_ps",
                                        tag=f"rot{tt % 2}")
                        for hf in range(2):
                            nc.tensor.matmul(
                                f_ps[:, hf * 512:(hf + 1) * 512],
                                outT_sb[:, tt * 128:(tt + 1) * 128],
                                W2_sb[:, hf * 512:(hf + 1) * 512],
                                start=True, stop=False)
                            # bias row: += sums[t] * fcb[h]
                            nc.tensor.matmul(
                                f_ps[:, hf * 512:(hf + 1) * 512],
                                sumsT_sb[0:1, tt * 128:(tt + 1) * 128],
                                fcb_sb[0:1, hf * 512:(hf + 1) * 512],
                                start=False, stop=True)
                        # normalized pre-selu F = recip[t] * f_ps
                        rc = recips[tt]
                        c1r = fpool.tile([128, 1], f32, name="c1r")
                        nc.vector.tensor_tensor(c1r[:], c1[:], rc[:],
                                                op=ALU.mult)
                        e2 = fpool.tile([128, H], f32, name="e2")
                        nc.scalar.activation(e2[:], f_ps[:], AF.Exp,
                                             bias=lnc2[:], scale=rc[:])
                        rr = fpool.tile([128, H], f32, name="rr")
                        nc.vector.tensor_scalar(rr[:], f_ps[:], 0.0, c1r[:],
                                                op0=ALU.max, op1=ALU.mult)
                        part = fpool.tile([128, H], bf16, name="part")
                        nc.vector.scalar_tensor_tensor(
                            part[:], e2[:], c2[:], rr[:],
                            op0=ALU.min, op1=ALU.add)
                        last_part_dma = nc.sync.dma_start(
                            cc_in[tt * 128:(tt + 1) * 128, :], part[:])

                    # ---------- phase F: launch ReduceScatter ----------
                    cc_out = dpool.tile([128, H], bf16, name=f"cc_out{b}",
                                        bufs=1)
                    nc.gpsimd.collective_compute(
                        "ReduceScatter", ALU.add,
                        replica_groups=[list(range(N_CORES))],
                        ins=[cc_in.opt()], outs=[cc_out.opt()])
                    cc_outs.append(cc_out)

                # ---------- deferred: residual + LN per batch ----------
                for b in range(B):
                    x_sb = lnpool.tile([128, H], bf16, name="x_sb", tag="lnbf", bufs=4)
                    x_dma = nc.sync.dma_start(x_sb[:], cc_outs[b][:])
                    add_dep_helper(x_dma.ins, last_part_dma.ins, sync=True,
                                   reason="defer LN past all compute")
                    qres_sb = lnpool.tile([128, H], f32, name="qres_sb", tag="lnbig", bufs=8)
                    nc.sync.dma_start(qres_sb[:], qres_d[b])
                    x2 = lnpool.tile([128, H], f32, name="x2", tag="lnbig", bufs=8)
                    xsum = lnpool.tile([128, 1], f32, name="xsum")
                    nc.vector.scalar_tensor_tensor(
                        x2[:], x_sb[:], 1.0, qres_sb[:],
                        op0=ALU.mult, op1=ALU.add, accum_out=xsum[:])
                    sqs = lnpool.tile([128, H], f32, name="sqs", tag="lnbig", bufs=8)
                    x2sum = lnpool.tile([128, 1], f32, name="x2sum")
                    nc.scalar.activation(sqs[:], x2[:], AF.Square,
                                         accum_out=x2sum[:])
                    negmu = lnpool.tile([128, 1], f32, name="negmu")
                    nc.vector.tensor_scalar(negmu[:], xsum[:], -1.0 / H, None,
                                            op0=ALU.mult)
                    varpe = lnpool.tile([128, 1], f32, name="varpe")
                    nc.vector.tensor_scalar(varpe[:], x2sum[:], 1.0 / H,
                                            LN_EPS, op0=ALU.mult, op1=ALU.add)
                    mu2 = lnpool.tile([128, 1], f32, name="mu2")
                    nc.vector.tensor_tensor(mu2[:], negmu[:], negmu[:],
                                            op=ALU.mult)
                    varr = lnpool.tile([128, 1], f32, name="varr")
                    nc.vector.tensor_tensor(varr[:], varpe[:], mu2[:],
                                            op=ALU.subtract)
                    rstd = rsqrt_newton(lnpool, varr, "rs")
                    xhat = lnpool.tile([128, H], f32, name="xhat", tag="lnbig", bufs=8)
                    nc.vector.tensor_scalar(xhat[:], x2[:], negmu[:], rstd[:],
                                            op0=ALU.add, op1=ALU.mult)
                    t1 = lnpool.tile([128, H], f32, name="t1", tag="lnbig", bufs=8)
                    nc.vector.tensor_tensor(t1[:], xhat[:], lng_bc[:],
                                            op=ALU.mult)
                    y_sb = lnpool.tile([128, H], f32, name="y_sb", tag="lnbig", bufs=8)
                    nc.vector.tensor_tensor(y_sb[:], t1[:], lnb_bc[:],
                                            op=ALU.add)
                    nc.sync.dma_start(y_d[b], y_sb[:])

    nc.compile()
    return nc


def _get_compiled():
    if "nc" not in _CACHE:
        _CACHE["nc"] = _build()
    return _CACHE["nc"]


def kernel(q, k, v, w_q, w_k, w_v, w_o, a_gate, k_gate, fc_w, fc_b,
           ln_gamma, ln_beta):
    import ml_dtypes
    from concourse.bass_utils import run_bass_kernel_spmd

    q = np.asarray(q, np.float32)
    k = np.asarray(k, np.float32)
    v = np.asarray(v, np.float32)
    w_q = np.asarray(w_q, np.float32)
    w_k = np.asarray(w_k, np.float32)
    w_v = np.asarray(w_v, np.float32)
    w_o = np.asarray(w_o, np.float32)
    a_gate = np.asarray(a_gate, np.float32)
    k_gate = np.asarray(k_gate, np.float32)
    fc_w = np.asarray(fc_w, np.float32)
    fc_b = np.asarray(fc_b, np.float32)
    ln_gamma = np.asarray(ln_gamma, np.float32)
    ln_beta = np.asarray(ln_beta, np.float32)

    nc = _get_compiled()

    qT = np.ascontiguousarray(q.reshape(B * T, H).T)
    kT = np.ascontiguousarray(k.reshape(B * T, H).T)
    vT = np.ascontiguousarray(v.reshape(B * T, H).T)
    fcb = np.ascontiguousarray(fc_b.reshape(1, H)).astype(ml_dtypes.bfloat16)
    ag = np.ascontiguousarray(a_gate.reshape(1, NH))
    kg = np.ascontiguousarray(k_gate.reshape(1, NH))
    lng = np.ascontiguousarray(ln_gamma.reshape(1, H))
    lnb = np.ascontiguousarray(ln_beta.reshape(1, H))

    in_maps = []
    for n in range(N_CORES):
        sel = np.zeros((1, NH), np.float32)
        sel[0, n] = 1.0
        in_maps.append({
            "qT": qT, "kT": kT, "vT": vT,
            "wq": np.ascontiguousarray(w_q[n]),
            "wk": np.ascontiguousarray(w_k[n]),
            "wv": np.ascontiguousarray(w_v[n]),
            "wo": np.ascontiguousarray(w_o[n]),
            "fcw": np.ascontiguousarray(fc_w).astype(ml_dtypes.bfloat16),
            "fcb": fcb,
            "ag": ag, "kg": kg, "sel": sel,
            "lng": lng, "lnb": lnb,
            "qres": np.ascontiguousarray(q[:, n * 128:(n + 1) * 128, :]),
        })

    res = run_bass_kernel_spmd(nc, in_maps, core_ids=list(range(N_CORES)))
    _CACHE["last_res"] = res

    y = np.empty((B, T, H), np.float32)
    attn = np.empty((NH * B, T, T), np.float32)
    for n in range(N_CORES):
        y[:, n * 128:(n + 1) * 128, :] = res.results[n]["y_out"]
        attn[n * B:(n + 1) * B] = res.results[n]["attn_out"]
    return y, attn


# revision 13
# speedup vs baseline: 1.1594x; 1.1594x over previous
"""BranchedAttention Trainium2 kernel (8-core head-parallel SPMD).

Strategy (head parallel per sharding hint):
  - core n owns head n: projections, attention, folded output projection.
  - w_o @ fc_w are back-to-back linear maps (selu comes after fc_w), so they
    are folded on device into W2[n] = softmax(k_gate)[n] * (w_o[n] @ fc_w).
  - scores computed twice (natural [q,s] for softmax+output, transposed [s,q]
    for the attn@v matmul) to avoid on-device fp32 transposes of attention.
    attn@v consumes unnormalized exp; the softmax denominator is folded in
    after the V matmul (linearity).
  - head-sum via ReduceScatter per batch; residual+layernorm on the shard.

Self-contained: hardcodes all shapes from the problem spec.
"""

import numpy as np

B = 4
T = 1024          # q_len == seq_len
H = 1024          # hidden
NH = 8            # heads
DK = 128          # per-head dim
E = 2 * H         # fc input dim
N_CORES = 8
LN_EPS = 1e-6
INV_SQRT_DK = 1.0 / float(np.sqrt(DK))
SELU_SCALE = 1.0507009873554805
SELU_ALPHA = 1.6732632423543772

_CACHE = {}


def _build():
    import concourse.bass as bass
    import concourse.mybir as mybir
    import concourse.tile as tile
    from concourse import bacc
    from concourse.masks import make_identity
    from concourse.tile_rust import add_dep_helper

    f32 = mybir.dt.float32
    f32r = mybir.dt.float32r
    bf16 = mybir.dt.bfloat16
    u32 = mybir.dt.uint32
    AF = mybir.ActivationFunctionType
    ALU = mybir.AluOpType

    nc = bacc.Bacc("TRN2", target_bir_lowering=False, debug=False,
                   num_devices=N_CORES)

    # ---- I/O ----
    qT_d = nc.dram_tensor("qT", [H, B * T], bf16, kind="ExternalInput")
    kT_d = nc.dram_tensor("kT", [H, B * T], bf16, kind="ExternalInput")
    vT_d = nc.dram_tensor("vT", [H, B * T], bf16, kind="ExternalInput")
    wq_d = nc.dram_tensor("wq", [H, DK], bf16, kind="ExternalInput")
    wk_d = nc.dram_tensor("wk", [H, DK], bf16, kind="ExternalInput")
    wv_d = nc.dram_tensor("wv", [H, DK], bf16, kind="ExternalInput")
    wo_d = nc.dram_tensor("wo", [DK, E], f32r, kind="ExternalInput")
    fcw_d = nc.dram_tensor("fcw", [E, H], bf16, kind="ExternalInput")
    fcb_d = nc.dram_tensor("fcb", [1, H], bf16, kind="ExternalInput")
    ag_d = nc.dram_tensor("ag", [1, NH], f32, kind="ExternalInput")
    kg_d = nc.dram_tensor("kg", [1, NH], f32, kind="ExternalInput")
    sel_d = nc.dram_tensor("sel", [1, NH], f32, kind="ExternalInput")
    lng_d = nc.dram_tensor("lng", [1, H], f32, kind="ExternalInput")
    lnb_d = nc.dram_tensor("lnb", [1, H], f32, kind="ExternalInput")
    qres_d = nc.dram_tensor("qres", [B, 128, H], f32, kind="ExternalInput")

    attn_d = nc.dram_tensor("attn_out", [B, T, T], f32, kind="ExternalOutput")
    y_d = nc.dram_tensor("y_out", [B, 128, H], f32, kind="ExternalOutput")

    def rsqrt_newton(pool, varr, name):
        """1/sqrt(varr) on [128,1] via quake initial guess + 3 Newton steps."""
        t0 = pool.tile([128, 1], u32, name=f"{name}_t0")
        nc.vector.tensor_scalar(t0[:], varr[:].bitcast(u32), 1, None,
                                op0=ALU.logical_shift_right)
        y0 = pool.tile([128, 1], f32, name=f"{name}_y0")
        nc.vector.tensor_tensor(y0[:].bitcast(u32), magic[:], t0[:],
                                op=ALU.subtract)
        ah = pool.tile([128, 1], f32, name=f"{name}_ah")
        nc.vector.tensor_scalar(ah[:], varr[:], 0.5, None, op0=ALU.mult)
        ycur = y0
        for it in range(3):
            sq = pool.tile([128, 1], f32, name=f"{name}_sq{it}")
            nc.vector.tensor_tensor(sq[:], ycur[:], ycur[:], op=ALU.mult)
            u = pool.tile([128, 1], f32, name=f"{name}_u{it}")
            nc.vector.tensor_tensor(u[:], sq[:], ah[:], op=ALU.mult)
            w2c = pool.tile([128, 1], f32, name=f"{name}_w2{it}")
            nc.vector.tensor_scalar(w2c[:], u[:], -1.0, 1.5,
                                    op0=ALU.mult, op1=ALU.add)
            yn = pool.tile([128, 1], f32, name=f"{name}_yn{it}")
            nc.vector.tensor_tensor(yn[:], ycur[:], w2c[:], op=ALU.mult)
            ycur = yn
        return ycur

    with tile.TileContext(nc) as tc:
        with (
            tc.tile_pool(name="const", bufs=1) as cpool,
            tc.tile_pool(name="wpool", bufs=1) as wpool,
        ):
            # ---- constants ----
            ident = cpool.tile([128, 128], f32)
            make_identity(nc, ident[:])
            magic = cpool.tile([128, 1], u32)
            nc.gpsimd.memset(magic[:], 0x5F3759DF)

            # ---- gate softmaxes; select this head's entries ----
            ag_t = cpool.tile([1, NH], f32)
            kg_t = cpool.tile([1, NH], f32)
            sel_t = cpool.tile([1, NH], f32)
            nc.sync.dma_start(ag_t[:], ag_d[:])
            nc.sync.dma_start(kg_t[:], kg_d[:])
            nc.sync.dma_start(sel_t[:], sel_d[:])

            def gate_scalar(gt, name):
                ge = cpool.tile([1, NH], f32, name=f"{name}_ge")
                gs = cpool.tile([1, 1], f32, name=f"{name}_gs")
                nc.scalar.activation(ge[:], gt[:], AF.Exp, accum_out=gs[:])
                gr = cpool.tile([1, 1], f32, name=f"{name}_gr")
                nc.vector.reciprocal(gr[:], gs[:])
                picked = cpool.tile([1, NH], f32, name=f"{name}_picked")
                nc.vector.tensor_tensor(picked[:], ge[:], sel_t[:], op=ALU.mult)
                psum_ = cpool.tile([1, 1], f32, name=f"{name}_psum")
                nc.vector.reduce_sum(psum_[:], picked[:],
                                     axis=mybir.AxisListType.X)
                out = cpool.tile([1, 1], f32, name=f"{name}_sm")
                nc.vector.tensor_tensor(out[:], psum_[:], gr[:], op=ALU.mult)
                return out

            sma = gate_scalar(ag_t, "a")     # softmax(a_gate)[n]
            smk = gate_scalar(kg_t, "k")     # softmax(k_gate)[n]
            sma_bc = cpool.tile([128, 1], f32)
            smk_bc = cpool.tile([128, 1], f32)
            nc.gpsimd.partition_broadcast(sma_bc[:], sma[:])
            nc.gpsimd.partition_broadcast(smk_bc[:], smk[:])
            c1 = cpool.tile([128, 1], f32)   # selu_scale * sm_a
            nc.vector.tensor_scalar(c1[:], sma_bc[:], SELU_SCALE, None,
                                    op0=ALU.mult)
            c2 = cpool.tile([128, 1], f32)   # selu_scale * alpha * sm_a
            nc.vector.tensor_scalar(c2[:], sma_bc[:], SELU_SCALE * SELU_ALPHA,
                                    None, op0=ALU.mult)
            lnc2 = cpool.tile([128, 1], f32)
            nc.scalar.activation(lnc2[:], c2[:], AF.Ln)

            # ---- LN gamma/beta broadcast ----
            lng_row = cpool.tile([1, H], f32)
            lnb_row = cpool.tile([1, H], f32)
            nc.sync.dma_start(lng_row[:], lng_d[:])
            nc.sync.dma_start(lnb_row[:], lnb_d[:])
            lng_bc = cpool.tile([128, H], f32)
            lnb_bc = cpool.tile([128, H], f32)
            nc.gpsimd.partition_broadcast(lng_bc[:], lng_row[:])
            nc.gpsimd.partition_broadcast(lnb_bc[:], lnb_row[:])

            # ---- per-head projection weights ----
            wq_sb = wpool.tile([128, H], bf16)   # col block h: wq[h*128:+128,:]
            wk_sb = wpool.tile([128, H], bf16)
            wv_sb = wpool.tile([128, H], bf16)
            for h in range(8):
                nc.sync.dma_start(wq_sb[:, h * 128:(h + 1) * 128],
                                  wq_d[h * 128:(h + 1) * 128, :])
                nc.sync.dma_start(wk_sb[:, h * 128:(h + 1) * 128],
                                  wk_d[h * 128:(h + 1) * 128, :])
                nc.sync.dma_start(wv_sb[:, h * 128:(h + 1) * 128],
                                  wv_d[h * 128:(h + 1) * 128, :])

            fcb_sb = cpool.tile([1, H], bf16)
            nc.sync.dma_start(fcb_sb[:], fcb_d[:])

            # ---- fold W2 = smk * (w_o @ fc_w)  [DK, H] ----
            W2_sb = wpool.tile([128, H], bf16)
            with (
                tc.tile_pool(name="stage", bufs=1) as stage,
                tc.tile_pool(name="stage_ps", bufs=2, space="PSUM") as stage_ps,
            ):
                wo_sb = stage.tile([128, E], f32r)
                nc.sync.dma_start(wo_sb[:], wo_d[:])
                # scale by softmax(k_gate)[n]
                nc.vector.tensor_scalar(wo_sb[:].bitcast(f32),
                                        wo_sb[:].bitcast(f32), smk_bc[:],
                                        None, op0=ALU.mult)
                # transpose wo -> woT [E, DK] (16 col blocks of [128,128])
                woT_sb = stage.tile([128, 16 * 128], bf16)
                for g in range(4):
                    tr_ps = stage_ps.tile([128, 512], f32, name="tr_ps")
                    for j in range(4):
                        e = g * 4 + j
                        nc.tensor.transpose(
                            tr_ps[:, j * 128:(j + 1) * 128],
                            wo_sb[:, e * 128:(e + 1) * 128].bitcast(f32),
                            ident[:])
                    nc.vector.tensor_copy(
                        woT_sb[:, g * 512:(g + 1) * 512], tr_ps[:])
                # fcw chunks + accumulate W2
                w2_halves = []
                for half in range(2):
                    w2_ps_h = stage_ps.tile([128, 512], f32,
                                            name=f"w2_ps{half}", bufs=1)
                    w2_halves.append(w2_ps_h)
                for e in range(16):
                    fcw_t = stage.tile([128, H], bf16, name="fcw_t", bufs=3)
                    nc.sync.dma_start(fcw_t[:],
                                      fcw_d[e * 128:(e + 1) * 128, :])
                    for half in range(2):
                        nc.tensor.matmul(
                            w2_halves[half][:],
                            woT_sb[:, e * 128:(e + 1) * 128],
                            fcw_t[:, half * 512:(half + 1) * 512],
                            start=(e == 0), stop=(e == 15))
                for half in range(2):
                    nc.vector.tensor_copy(
                        W2_sb[:, half * 512:(half + 1) * 512],
                        w2_halves[half][:])

            # ---- main pools ----
            with (
                tc.tile_pool(name="stream", bufs=3) as stream,
                tc.tile_pool(name="proj", bufs=2) as proj,
                tc.tile_pool(name="apool", bufs=2) as apool,
                tc.tile_pool(name="fpool", bufs=2) as fpool,
                tc.tile_pool(name="lnpool", bufs=2) as lnpool,
                tc.tile_pool(name="mps", bufs=1, space="PSUM") as mps,
                tc.tile_pool(name="dpool", bufs=2, space="DRAM") as dpool,
            ):
                cc_outs = []
                for b in range(B):
                    c0 = b * T  # column offset of this batch in [H, B*T]

                    # ---------- phase A: projections ----------
                    qh_ps = mps.tile([128, T], f32, name="qh_ps", tag="acc1")
                    kh_ps = mps.tile([128, T], f32, name="kh_ps", tag="acc2")
                    for h in range(8):
                        qch = stream.tile([128, T], bf16, name="qch")
                        nc.sync.dma_start(
                            qch[:], qT_d[h * 128:(h + 1) * 128, c0:c0 + T])
                        for hf in range(2):
                            nc.tensor.matmul(
                                qh_ps[:, hf * 512:(hf + 1) * 512],
                                wq_sb[:, h * 128:(h + 1) * 128],
                                qch[:, hf * 512:(hf + 1) * 512],
                                start=(h == 0), stop=(h == 7))
                        kch = stream.tile([128, T], bf16, name="kch")
                        nc.sync.dma_start(
                            kch[:], kT_d[h * 128:(h + 1) * 128, c0:c0 + T])
                        for hf in range(2):
                            nc.tensor.matmul(
                                kh_ps[:, hf * 512:(hf + 1) * 512],
                                wk_sb[:, h * 128:(h + 1) * 128],
                                kch[:, hf * 512:(hf + 1) * 512],
                                start=(h == 0), stop=(h == 7))
                    qhT_sb = proj.tile([128, T], f32r, name="qhT_sb")
                    khT_sb = proj.tile([128, T], f32r, name="khT_sb")
                    nc.vector.tensor_copy(qhT_sb[:], qh_ps[:])
                    nc.scalar.copy(khT_sb[:], kh_ps[:])
                    vh_ps = mps.tile([128, T], f32, name="vh_ps", tag="acc1")
                    for h in range(8):
                        vch = stream.tile([128, T], bf16, name="vch")
                        nc.sync.dma_start(
                            vch[:], vT_d[h * 128:(h + 1) * 128, c0:c0 + T])
                        for hf in range(2):
                            nc.tensor.matmul(
                                vh_ps[:, hf * 512:(hf + 1) * 512],
                                wv_sb[:, h * 128:(h + 1) * 128],
                                vch[:, hf * 512:(hf + 1) * 512],
                                start=(h == 0), stop=(h == 7))
                    # vhT [d, s] -> transpose to vh natural [s, d] col blocks
                    vhT_sb = proj.tile([128, T], f32, name="vhT_sb")
                    nc.vector.tensor_copy(vhT_sb[:], vh_ps[:])
                    vh_sb = proj.tile([128, T], bf16, name="vh_sb")
                    for g in range(2):
                        tr_ps = mps.tile([128, 512], f32, name="tr_ps",
                                         tag=f"rot{g}")
                        for j in range(4):
                            st = g * 4 + j
                            nc.tensor.transpose(
                                tr_ps[:, j * 128:(j + 1) * 128],
                                vhT_sb[:, st * 128:(st + 1) * 128],
                                ident[:])
                        nc.vector.tensor_copy(
                            vh_sb[:, g * 512:(g + 1) * 512], tr_ps[:])

                    # ---------- phase B: scores + softmax (natural) ----------
                    sumsT_sb = apool.tile([1, T], bf16, name="sumsT_sb")
                    recips = []
                    for qt in range(8):
                        sc_ps = mps.tile([128, T], f32, name="sc_ps",
                                         tag=f"rot{qt % 2}")
                        for hf in range(2):
                            nc.tensor.matmul(
                                sc_ps[:, hf * 512:(hf + 1) * 512],
                                qhT_sb[:, qt * 128:(qt + 1) * 128],
                                khT_sb[:, hf * 512:(hf + 1) * 512],
                                start=True, stop=True)
                        exp_sb = apool.tile([128, T], f32, name="exp_sb")
                        sums = apool.tile([128, 1], f32, name="sums")
                        nc.scalar.activation(exp_sb[:], sc_ps[:], AF.Exp,
                                             scale=INV_SQRT_DK,
                                             accum_out=sums[:])
                        recip = apool.tile([128, 1], f32, name="recip",
                                           bufs=12)
                        recips.append(recip)
                        nc.vector.reciprocal(recip[:], sums[:])
                        nc.vector.tensor_scalar(exp_sb[:], exp_sb[:],
                                                recip[:], None, op0=ALU.mult)
                        nc.sync.dma_start(
                            attn_d[b, qt * 128:(qt + 1) * 128, :], exp_sb[:])
                        # sums -> transposed [1,128] into sumsT_sb (bias row)
                        rtp = mps.tile([1, 128], f32, name="rtp", tag="acc2")
                        nc.tensor.transpose(rtp[:], sums[:], ident[:])
                        nc.vector.tensor_copy(
                            sumsT_sb[0:1, qt * 128:(qt + 1) * 128], rtp[:])

                    # ---------- phase C+D: scoresT + exp, attn @ v ----------
                    outT_ps = mps.tile([128, T], f32, name="outT_ps",
                                       tag="acc1")
                    for st in range(8):
                        scT_ps = mps.tile([128, T], f32, name="scT_ps",
                                          tag=f"rot{st % 2}")
                        for hf in range(2):
                            nc.tensor.matmul(
                                scT_ps[:, hf * 512:(hf + 1) * 512],
                                khT_sb[:, st * 128:(st + 1) * 128],
                                qhT_sb[:, hf * 512:(hf + 1) * 512],
                                start=True, stop=True)
                        expT = apool.tile([128, T], bf16, name="expT", bufs=2)
                        nc.scalar.activation(expT[:], scT_ps[:],
                                             AF.Exp, scale=INV_SQRT_DK)
                        for hf in range(2):
                            nc.tensor.matmul(
                                outT_ps[:, hf * 512:(hf + 1) * 512],
                                vh_sb[:, st * 128:(st + 1) * 128],
                                expT[:, hf * 512:(hf + 1) * 512],
                                start=(st == 0), stop=(st == 7))
                    outT_sb = proj.tile([128, T], bf16, name="outT_sb")
                    nc.vector.tensor_copy(outT_sb[:], outT_ps[:])

                    # ---------- phase E: f = outT.T @ W2 + fcb; selu; gate ---
                    cc_in = dpool.tile([T, H], bf16, name="cc_in")
                    for tt in range(8):
                        f_ps = mps.tile([128, H], f32, name="f_ps",
                                        tag=f"rot{tt % 2}")
                        for hf in range(2):
                            nc.tensor.matmul(
                                f_ps[:, hf * 512:(hf + 1) * 512],
                                outT_sb[:, tt * 128:(tt + 1) * 128],
                                W2_sb[:, hf * 512:(hf + 1) * 512],
                                start=True, stop=False)
                            # bias row: += sums[t] * fcb[h]
                            nc.tensor.matmul(
                                f_ps[:, hf * 512:(hf + 1) * 512],
                                sumsT_sb[0:1, tt * 128:(tt + 1) * 128],
                                fcb_sb[0:1, hf * 512:(hf + 1) * 512],
                                start=False, stop=True)
                        # normalized pre-selu F = recip[t] * f_ps
                        rc = recips[tt]
                        c1r = fpool.tile([128, 1], f32, name="c1r")
                        nc.vector.tensor_tensor(c1r[:], c1[:], rc[:],
                                                op=ALU.mult)
                        e2 = fpool.tile([128, H], f32, name="e2")
                        nc.scalar.activation(e2[:], f_ps[:], AF.Exp,
                                             bias=lnc2[:], scale=rc[:])
                        rr = fpool.tile([128, H], f32, name="rr")
                        nc.vector.tensor_scalar(rr[:], f_ps[:], 0.0, c1r[:],
                                                op0=ALU.max, op1=ALU.mult)
                        part = fpool.tile([128, H], bf16, name="part")
                        nc.vector.scalar_tensor_tensor(
                            part[:], e2[:], c2[:], rr[:],
                            op0=ALU.min, op1=ALU.add)
                        last_part_dma = nc.sync.dma_start(
                            cc_in[tt * 128:(tt + 1) * 128, :], part[:])

                    # ---------- phase F: launch ReduceScatter ----------
                    cc_out = dpool.tile([128, H], bf16, name=f"cc_out{b}",
                                        bufs=1)
                    nc.gpsimd.collective_compute(
                        "ReduceScatter", ALU.add,
                        replica_groups=[list(range(N_CORES))],
                        ins=[cc_in.opt()], outs=[cc_out.opt()])
                    cc_outs.append(cc_out)

                # ---------- deferred: residual + LN per batch ----------
                for b in range(B):
                    x_sb = lnpool.tile([128, H], bf16, name="x_sb", tag="lnbf", bufs=4)
                    x_dma = nc.sync.dma_start(x_sb[:], cc_outs[b][:])
                    add_dep_helper(x_dma.ins, last_part_dma.ins, sync=True,
                                   reason="defer LN past all compute")
                    qres_sb = lnpool.tile([128, H], f32, name="qres_sb", tag="lnbig", bufs=8)
                    nc.sync.dma_start(qres_sb[:], qres_d[b])
                    x2 = lnpool.tile([128, H], f32, name="x2", tag="lnbig", bufs=8)
                    xsum = lnpool.tile([128, 1], f32, name="xsum")
                    nc.vector.scalar_tensor_tensor(
                        x2[:], x_sb[:], 1.0, qres_sb[:],
                        op0=ALU.mult, op1=ALU.add, accum_out=xsum[:])
                    sqs = lnpool.tile([128, H], f32, name="sqs", tag="lnbig", bufs=8)
                    x2sum = lnpool.tile([128, 1], f32, name="x2sum")
                    nc.scalar.activation(sqs[:], x2[:], AF.Square,
                                         accum_out=x2sum[:])
                    negmu = lnpool.tile([128, 1], f32, name="negmu")
                    nc.vector.tensor_scalar(negmu[:], xsum[:], -1.0 / H, None,
                                            op0=ALU.mult)
                    varpe = lnpool.tile([128, 1], f32, name="varpe")
                    nc.vector.tensor_scalar(varpe[:], x2sum[:], 1.0 / H,
                                            LN_EPS, op0=ALU.mult, op1=ALU.add)
                    mu2 = lnpool.tile([128, 1], f32, name="mu2")
                    nc.vector.tensor_tensor(mu2[:], negmu[:], negmu[:],
                                            op=ALU.mult)
                    varr = lnpool.tile([128, 1], f32, name="varr")
                    nc.vector.tensor_tensor(varr[:], varpe[:], mu2[:],
                                            op=ALU.subtract)
                    rstd = rsqrt_newton(lnpool, varr, "rs")
                    xhat = lnpool.tile([128, H], f32, name="xhat", tag="lnbig", bufs=8)
                    nc.vector.tensor_scalar(xhat[:], x2[:], negmu[:], rstd[:],
                                            op0=ALU.add, op1=ALU.mult)
                    t1 = lnpool.tile([128, H], f32, name="t1", tag="lnbig", bufs=8)
                    nc.vector.tensor_tensor(t1[:], xhat[:], lng_bc[:],
                                            op=ALU.mult)
                    y_sb = lnpool.tile([128, H], f32, name="y_sb", tag="lnbig", bufs=8)
                    nc.vector.tensor_tensor(y_sb[:], t1[:], lnb_bc[:],
                                            op=ALU.add)
                    nc.sync.dma_start(y_d[b], y_sb[:])

    nc.compile()
    return nc


def _get_compiled():
    if "nc" not in _CACHE:
        _CACHE["nc"] = _build()
    return _CACHE["nc"]


def kernel(q, k, v, w_q, w_k, w_v, w_o, a_gate, k_gate, fc_w, fc_b,
           ln_gamma, ln_beta):
    import ml_dtypes
    from concourse.bass_utils import run_bass_kernel_spmd

    q = np.asarray(q, np.float32)
    k = np.asarray(k, np.float32)
    v = np.asarray(v, np.float32)
    w_q = np.asarray(w_q, np.float32)
    w_k = np.asarray(w_k, np.float32)
    w_v = np.asarray(w_v, np.float32)
    w_o = np.asarray(w_o, np.float32)
    a_gate = np.asarray(a_gate, np.float32)
    k_gate = np.asarray(k_gate, np.float32)
    fc_w = np.asarray(fc_w, np.float32)
    fc_b = np.asarray(fc_b, np.float32)
    ln_gamma = np.asarray(ln_gamma, np.float32)
    ln_beta = np.asarray(ln_beta, np.float32)

    nc = _get_compiled()

    qT = np.ascontiguousarray(q.reshape(B * T, H).T).astype(ml_dtypes.bfloat16)
    kT = np.ascontiguousarray(k.reshape(B * T, H).T).astype(ml_dtypes.bfloat16)
    vT = np.ascontiguousarray(v.reshape(B * T, H).T).astype(ml_dtypes.bfloat16)
    fcb = np.ascontiguousarray(fc_b.reshape(1, H)).astype(ml_dtypes.bfloat16)
    ag = np.ascontiguousarray(a_gate.reshape(1, NH))
    kg = np.ascontiguousarray(k_gate.reshape(1, NH))
    lng = np.ascontiguousarray(ln_gamma.reshape(1, H))
    lnb = np.ascontiguousarray(ln_beta.reshape(1, H))

    in_maps = []
    for n in range(N_CORES):
        sel = np.zeros((1, NH), np.float32)
        sel[0, n] = 1.0
        in_maps.append({
            "qT": qT, "kT": kT, "vT": vT,
            "wq": np.ascontiguousarray(w_q[n]).astype(ml_dtypes.bfloat16),
            "wk": np.ascontiguousarray(w_k[n]).astype(ml_dtypes.bfloat16),
            "wv": np.ascontiguousarray(w_v[n]).astype(ml_dtypes.bfloat16),
            "wo": np.ascontiguousarray(w_o[n]),
            "fcw": np.ascontiguousarray(fc_w).astype(ml_dtypes.bfloat16),
            "fcb": fcb,
            "ag": ag, "kg": kg, "sel": sel,
            "lng": lng, "lnb": lnb,
            "qres": np.ascontiguousarray(q[:, n * 128:(n + 1) * 128, :]),
        })

    res = run_bass_kernel_spmd(nc, in_maps, core_ids=list(range(N_CORES)))
    _CACHE["last_res"] = res

    y = np.empty((B, T, H), np.float32)
    attn = np.empty((NH * B, T, T), np.float32)
    for n in range(N_CORES):
        y[:, n * 128:(n + 1) * 128, :] = res.results[n]["y_out"]
        attn[n * B:(n + 1) * B] = res.results[n]["attn_out"]
    return y, attn


# revision 14
# speedup vs baseline: 1.2530x; 1.0807x over previous
"""BranchedAttention Trainium2 kernel (8-core head-parallel SPMD).

Strategy (head parallel per sharding hint):
  - core n owns head n: projections, attention, folded output projection.
  - w_o @ fc_w are back-to-back linear maps (selu comes after fc_w), so they
    are folded on device into W2[n] = softmax(k_gate)[n] * (w_o[n] @ fc_w).
  - scores computed twice (natural [q,s] for softmax+output, transposed [s,q]
    for the attn@v matmul) to avoid on-device fp32 transposes of attention.
    attn@v consumes unnormalized exp; the softmax denominator is folded in
    after the V matmul (linearity).
  - head-sum via ReduceScatter per batch; residual+layernorm on the shard.

Self-contained: hardcodes all shapes from the problem spec.
"""

import numpy as np

B = 4
T = 1024          # q_len == seq_len
H = 1024          # hidden
NH = 8            # heads
DK = 128          # per-head dim
E = 2 * H         # fc input dim
N_CORES = 8
LN_EPS = 1e-6
INV_SQRT_DK = 1.0 / float(np.sqrt(DK))
SELU_SCALE = 1.0507009873554805
SELU_ALPHA = 1.6732632423543772

_CACHE = {}


def _build():
    import concourse.bass as bass
    import concourse.mybir as mybir
    import concourse.tile as tile
    from concourse import bacc
    from concourse.masks import make_identity
    from concourse.tile_rust import add_dep_helper

    f32 = mybir.dt.float32
    f32r = mybir.dt.float32r
    bf16 = mybir.dt.bfloat16
    u32 = mybir.dt.uint32
    AF = mybir.ActivationFunctionType
    ALU = mybir.AluOpType

    nc = bacc.Bacc("TRN2", target_bir_lowering=False, debug=False,
                   num_devices=N_CORES)

    # ---- I/O ----
    qT_d = nc.dram_tensor("qT", [H, B * T], bf16, kind="ExternalInput")
    kT_d = nc.dram_tensor("kT", [H, B * T], bf16, kind="ExternalInput")
    vT_d = nc.dram_tensor("vT", [H, B * T], bf16, kind="ExternalInput")
    wq_d = nc.dram_tensor("wq", [H, DK], bf16, kind="ExternalInput")
    wk_d = nc.dram_tensor("wk", [H, DK], bf16, kind="ExternalInput")
    wv_d = nc.dram_tensor("wv", [H, DK], bf16, kind="ExternalInput")
    wo_d = nc.dram_tensor("wo", [DK, E], f32r, kind="ExternalInput")
    fcw_d = nc.dram_tensor("fcw", [E, H], bf16, kind="ExternalInput")
    fcb_d = nc.dram_tensor("fcb", [1, H], bf16, kind="ExternalInput")
    ag_d = nc.dram_tensor("ag", [1, NH], f32, kind="ExternalInput")
    kg_d = nc.dram_tensor("kg", [1, NH], f32, kind="ExternalInput")
    sel_d = nc.dram_tensor("sel", [1, NH], f32, kind="ExternalInput")
    lng_d = nc.dram_tensor("lng", [1, H], f32, kind="ExternalInput")
    lnb_d = nc.dram_tensor("lnb", [1, H], f32, kind="ExternalInput")
    qres_d = nc.dram_tensor("qres", [B, 128, H], f32, kind="ExternalInput")

    attn_d = nc.dram_tensor("attn_out", [B, T, T], f32, kind="ExternalOutput")
    y_d = nc.dram_tensor("y_out", [B, 128, H], f32, kind="ExternalOutput")

    def rsqrt_newton(pool, varr, name):
        """1/sqrt(varr) on [128,1] via quake initial guess + 3 Newton steps."""
        t0 = pool.tile([128, 1], u32, name=f"{name}_t0")
        nc.vector.tensor_scalar(t0[:], varr[:].bitcast(u32), 1, None,
                                op0=ALU.logical_shift_right)
        y0 = pool.tile([128, 1], f32, name=f"{name}_y0")
        nc.vector.tensor_tensor(y0[:].bitcast(u32), magic[:], t0[:],
                                op=ALU.subtract)
        ah = pool.tile([128, 1], f32, name=f"{name}_ah")
        nc.vector.tensor_scalar(ah[:], varr[:], 0.5, None, op0=ALU.mult)
        ycur = y0
        for it in range(3):
            sq = pool.tile([128, 1], f32, name=f"{name}_sq{it}")
            nc.vector.tensor_tensor(sq[:], ycur[:], ycur[:], op=ALU.mult)
            u = pool.tile([128, 1], f32, name=f"{name}_u{it}")
            nc.vector.tensor_tensor(u[:], sq[:], ah[:], op=ALU.mult)
            w2c = pool.tile([128, 1], f32, name=f"{name}_w2{it}")
            nc.vector.tensor_scalar(w2c[:], u[:], -1.0, 1.5,
                                    op0=ALU.mult, op1=ALU.add)
            yn = pool.tile([128, 1], f32, name=f"{name}_yn{it}")
            nc.vector.tensor_tensor(yn[:], ycur[:], w2c[:], op=ALU.mult)
            ycur = yn
        return ycur

    with tile.TileContext(nc) as tc:
        with (
            tc.tile_pool(name="const", bufs=1) as cpool,
            tc.tile_pool(name="wpool", bufs=1) as wpool,
        ):
            # ---- constants ----
            ident = cpool.tile([128, 128], f32)
            make_identity(nc, ident[:])
            magic = cpool.tile([128, 1], u32)
            nc.gpsimd.memset(magic[:], 0x5F3759DF)

            # ---- gate softmaxes; select this head's entries ----
            ag_t = cpool.tile([1, NH], f32)
            kg_t = cpool.tile([1, NH], f32)
            sel_t = cpool.tile([1, NH], f32)
            nc.sync.dma_start(ag_t[:], ag_d[:])
            nc.sync.dma_start(kg_t[:], kg_d[:])
            nc.sync.dma_start(sel_t[:], sel_d[:])

            def gate_scalar(gt, name):
                ge = cpool.tile([1, NH], f32, name=f"{name}_ge")
                gs = cpool.tile([1, 1], f32, name=f"{name}_gs")
                nc.scalar.activation(ge[:], gt[:], AF.Exp, accum_out=gs[:])
                gr = cpool.tile([1, 1], f32, name=f"{name}_gr")
                nc.vector.reciprocal(gr[:], gs[:])
                picked = cpool.tile([1, NH], f32, name=f"{name}_picked")
                nc.vector.tensor_tensor(picked[:], ge[:], sel_t[:], op=ALU.mult)
                psum_ = cpool.tile([1, 1], f32, name=f"{name}_psum")
                nc.vector.reduce_sum(psum_[:], picked[:],
                                     axis=mybir.AxisListType.X)
                out = cpool.tile([1, 1], f32, name=f"{name}_sm")
                nc.vector.tensor_tensor(out[:], psum_[:], gr[:], op=ALU.mult)
                return out

            sma = gate_scalar(ag_t, "a")     # softmax(a_gate)[n]
            smk = gate_scalar(kg_t, "k")     # softmax(k_gate)[n]
            sma_bc = cpool.tile([128, 1], f32)
            smk_bc = cpool.tile([128, 1], f32)
            nc.gpsimd.partition_broadcast(sma_bc[:], sma[:])
            nc.gpsimd.partition_broadcast(smk_bc[:], smk[:])
            c1 = cpool.tile([128, 1], f32)   # selu_scale * sm_a
            nc.vector.tensor_scalar(c1[:], sma_bc[:], SELU_SCALE, None,
                                    op0=ALU.mult)
            c2 = cpool.tile([128, 1], f32)   # selu_scale * alpha * sm_a
            nc.vector.tensor_scalar(c2[:], sma_bc[:], SELU_SCALE * SELU_ALPHA,
                                    None, op0=ALU.mult)
            lnc2 = cpool.tile([128, 1], f32)
            nc.scalar.activation(lnc2[:], c2[:], AF.Ln)

            # ---- LN gamma/beta broadcast ----
            lng_row = cpool.tile([1, H], f32)
            lnb_row = cpool.tile([1, H], f32)
            nc.sync.dma_start(lng_row[:], lng_d[:])
            nc.sync.dma_start(lnb_row[:], lnb_d[:])
            lng_bc = cpool.tile([128, H], f32)
            lnb_bc = cpool.tile([128, H], f32)
            nc.gpsimd.partition_broadcast(lng_bc[:], lng_row[:])
            nc.gpsimd.partition_broadcast(lnb_bc[:], lnb_row[:])

            # ---- per-head projection weights ----
            wq_sb = wpool.tile([128, H], bf16)   # col block h: wq[h*128:+128,:]
            wk_sb = wpool.tile([128, H], bf16)
            wv_sb = wpool.tile([128, H], bf16)
            for h in range(8):
                nc.sync.dma_start(wq_sb[:, h * 128:(h + 1) * 128],
                                  wq_d[h * 128:(h + 1) * 128, :])
                nc.sync.dma_start(wk_sb[:, h * 128:(h + 1) * 128],
                                  wk_d[h * 128:(h + 1) * 128, :])
                nc.sync.dma_start(wv_sb[:, h * 128:(h + 1) * 128],
                                  wv_d[h * 128:(h + 1) * 128, :])

            fcb_sb = cpool.tile([1, H], bf16)
            nc.sync.dma_start(fcb_sb[:], fcb_d[:])

            # ---- fold W2 = smk * (w_o @ fc_w)  [DK, H] ----
            W2_sb = wpool.tile([128, H], bf16)
            with (
                tc.tile_pool(name="stage", bufs=1) as stage,
                tc.tile_pool(name="stage_ps", bufs=2, space="PSUM") as stage_ps,
            ):
                wo_sb = stage.tile([128, E], f32r)
                nc.sync.dma_start(wo_sb[:], wo_d[:])
                # scale by softmax(k_gate)[n]
                nc.vector.tensor_scalar(wo_sb[:].bitcast(f32),
                                        wo_sb[:].bitcast(f32), smk_bc[:],
                                        None, op0=ALU.mult)
                # transpose wo -> woT [E, DK] (16 col blocks of [128,128])
                woT_sb = stage.tile([128, 16 * 128], bf16)
                for g in range(4):
                    tr_ps = stage_ps.tile([128, 512], f32, name="tr_ps")
                    for j in range(4):
                        e = g * 4 + j
                        nc.tensor.transpose(
                            tr_ps[:, j * 128:(j + 1) * 128],
                            wo_sb[:, e * 128:(e + 1) * 128].bitcast(f32),
                            ident[:])
                    nc.vector.tensor_copy(
                        woT_sb[:, g * 512:(g + 1) * 512], tr_ps[:])
                # fcw chunks + accumulate W2
                w2_halves = []
                for half in range(2):
                    w2_ps_h = stage_ps.tile([128, 512], f32,
                                            name=f"w2_ps{half}", bufs=1)
                    w2_halves.append(w2_ps_h)
                for e in range(16):
                    fcw_t = stage.tile([128, H], bf16, name="fcw_t", bufs=3)
                    nc.sync.dma_start(fcw_t[:],
                                      fcw_d[e * 128:(e + 1) * 128, :])
                    for half in range(2):
                        nc.tensor.matmul(
                            w2_halves[half][:],
                            woT_sb[:, e * 128:(e + 1) * 128],
                            fcw_t[:, half * 512:(half + 1) * 512],
                            start=(e == 0), stop=(e == 15))
                for half in range(2):
                    nc.vector.tensor_copy(
                        W2_sb[:, half * 512:(half + 1) * 512],
                        w2_halves[half][:])

            # ---- main pools ----
            with (
                tc.tile_pool(name="stream", bufs=6) as stream,
                tc.tile_pool(name="proj", bufs=2) as proj,
                tc.tile_pool(name="apool", bufs=2) as apool,
                tc.tile_pool(name="fpool", bufs=2) as fpool,
                tc.tile_pool(name="lnpool", bufs=2) as lnpool,
                tc.tile_pool(name="mps", bufs=1, space="PSUM") as mps,
                tc.tile_pool(name="dpool", bufs=2, space="DRAM") as dpool,
            ):
                cc_outs = []
                for b in range(B):
                    c0 = b * T  # column offset of this batch in [H, B*T]

                    # ---------- phase A: projections ----------
                    qh_ps = mps.tile([128, T], f32, name="qh_ps", tag="acc1")
                    kh_ps = mps.tile([128, T], f32, name="kh_ps", tag="acc2")
                    for h in range(8):
                        qch = stream.tile([128, T], bf16, name="qch")
                        nc.sync.dma_start(
                            qch[:], qT_d[h * 128:(h + 1) * 128, c0:c0 + T])
                        for hf in range(2):
                            nc.tensor.matmul(
                                qh_ps[:, hf * 512:(hf + 1) * 512],
                                wq_sb[:, h * 128:(h + 1) * 128],
                                qch[:, hf * 512:(hf + 1) * 512],
                                start=(h == 0), stop=(h == 7))
                        kch = stream.tile([128, T], bf16, name="kch")
                        nc.sync.dma_start(
                            kch[:], kT_d[h * 128:(h + 1) * 128, c0:c0 + T])
                        for hf in range(2):
                            nc.tensor.matmul(
                                kh_ps[:, hf * 512:(hf + 1) * 512],
                                wk_sb[:, h * 128:(h + 1) * 128],
                                kch[:, hf * 512:(hf + 1) * 512],
                                start=(h == 0), stop=(h == 7))
                    qhT_sb = proj.tile([128, T], f32r, name="qhT_sb")
                    khT_sb = proj.tile([128, T], f32r, name="khT_sb")
                    nc.vector.tensor_copy(qhT_sb[:], qh_ps[:])
                    nc.scalar.copy(khT_sb[:], kh_ps[:])
                    vh_ps = mps.tile([128, T], f32, name="vh_ps", tag="acc1")
                    for h in range(8):
                        vch = stream.tile([128, T], bf16, name="vch")
                        nc.sync.dma_start(
                            vch[:], vT_d[h * 128:(h + 1) * 128, c0:c0 + T])
                        for hf in range(2):
                            nc.tensor.matmul(
                                vh_ps[:, hf * 512:(hf + 1) * 512],
                                wv_sb[:, h * 128:(h + 1) * 128],
                                vch[:, hf * 512:(hf + 1) * 512],
                                start=(h == 0), stop=(h == 7))
                    # vhT [d, s] -> transpose to vh natural [s, d] col blocks
                    vhT_sb = proj.tile([128, T], f32, name="vhT_sb")
                    nc.vector.tensor_copy(vhT_sb[:], vh_ps[:])
                    vh_sb = proj.tile([128, T], bf16, name="vh_sb")
                    for g in range(2):
                        tr_ps = mps.tile([128, 512], f32, name="tr_ps",
                                         tag=f"rot{g}")
                        for j in range(4):
                            st = g * 4 + j
                            nc.tensor.transpose(
                                tr_ps[:, j * 128:(j + 1) * 128],
                                vhT_sb[:, st * 128:(st + 1) * 128],
                                ident[:])
                        nc.vector.tensor_copy(
                            vh_sb[:, g * 512:(g + 1) * 512], tr_ps[:])

                    # ---------- phase B: scores + softmax (natural) ----------
                    sumsT_sb = apool.tile([1, T], bf16, name="sumsT_sb")
                    recips = []
                    for qt in range(8):
                        sc_ps = mps.tile([128, T], f32, name="sc_ps",
                                         tag=f"rot{qt % 2}")
                        for hf in range(2):
                            nc.tensor.matmul(
                                sc_ps[:, hf * 512:(hf + 1) * 512],
                                qhT_sb[:, qt * 128:(qt + 1) * 128],
                                khT_sb[:, hf * 512:(hf + 1) * 512],
                                start=True, stop=True)
                        exp_sb = apool.tile([128, T], f32, name="exp_sb", bufs=3)
                        sums = apool.tile([128, 1], f32, name="sums")
                        nc.scalar.activation(exp_sb[:], sc_ps[:], AF.Exp,
                                             scale=INV_SQRT_DK,
                                             accum_out=sums[:])
                        recip = apool.tile([128, 1], f32, name="recip",
                                           bufs=12)
                        recips.append(recip)
                        nc.vector.reciprocal(recip[:], sums[:])
                        nc.vector.tensor_scalar(exp_sb[:], exp_sb[:],
                                                recip[:], None, op0=ALU.mult)
                        nc.sync.dma_start(
                            attn_d[b, qt * 128:(qt + 1) * 128, :], exp_sb[:])
                        # sums -> transposed [1,128] into sumsT_sb (bias row)
                        rtp = mps.tile([1, 128], f32, name="rtp", tag="acc2")
                        nc.tensor.transpose(rtp[:], sums[:], ident[:])
                        nc.vector.tensor_copy(
                            sumsT_sb[0:1, qt * 128:(qt + 1) * 128], rtp[:])

                    # ---------- phase C+D: scoresT + exp, attn @ v ----------
                    outT_ps = mps.tile([128, T], f32, name="outT_ps",
                                       tag="acc1")
                    for st in range(8):
                        scT_ps = mps.tile([128, T], f32, name="scT_ps",
                                          tag=f"rot{st % 2}")
                        for hf in range(2):
                            nc.tensor.matmul(
                                scT_ps[:, hf * 512:(hf + 1) * 512],
                                khT_sb[:, st * 128:(st + 1) * 128],
                                qhT_sb[:, hf * 512:(hf + 1) * 512],
                                start=True, stop=True)
                        expT = apool.tile([128, T], bf16, name="expT", bufs=3)
                        nc.scalar.activation(expT[:], scT_ps[:],
                                             AF.Exp, scale=INV_SQRT_DK)
                        for hf in range(2):
                            nc.tensor.matmul(
                                outT_ps[:, hf * 512:(hf + 1) * 512],
                                vh_sb[:, st * 128:(st + 1) * 128],
                                expT[:, hf * 512:(hf + 1) * 512],
                                start=(st == 0), stop=(st == 7))
                    outT_sb = proj.tile([128, T], bf16, name="outT_sb")
                    nc.vector.tensor_copy(outT_sb[:], outT_ps[:])

                    # ---------- phase E: f = outT.T @ W2 + fcb; selu; gate ---
                    cc_in = dpool.tile([T, H], bf16, name="cc_in")
                    for tt in range(8):
                        f_ps = mps.tile([128, H], f32, name="f_ps",
                                        tag=f"rot{tt % 2}")
                        for hf in range(2):
                            nc.tensor.matmul(
                                f_ps[:, hf * 512:(hf + 1) * 512],
                                outT_sb[:, tt * 128:(tt + 1) * 128],
                                W2_sb[:, hf * 512:(hf + 1) * 512],
                                start=True, stop=False)
                            # bias row: += sums[t] * fcb[h]
                            nc.tensor.matmul(
                                f_ps[:, hf * 512:(hf + 1) * 512],
                                sumsT_sb[0:1, tt * 128:(tt + 1) * 128],
                                fcb_sb[0:1, hf * 512:(hf + 1) * 512],
                                start=False, stop=True)
                        # normalized pre-selu F = recip[t] * f_ps
                        rc = recips[tt]
                        c1r = fpool.tile([128, 1], f32, name="c1r")
                        nc.vector.tensor_tensor(c1r[:], c1[:], rc[:],
                                                op=ALU.mult)
                        e2 = fpool.tile([128, H], f32, name="e2")
                        nc.scalar.activation(e2[:], f_ps[:], AF.Exp,
                                             bias=lnc2[:], scale=rc[:])
                        rr = fpool.tile([128, H], f32, name="rr")
                        nc.vector.tensor_scalar(rr[:], f_ps[:], 0.0, c1r[:],
                                                op0=ALU.max, op1=ALU.mult)
                        part = fpool.tile([128, H], bf16, name="part")
                        nc.vector.scalar_tensor_tensor(
                            part[:], e2[:], c2[:], rr[:],
                            op0=ALU.min, op1=ALU.add)
                        last_part_dma = nc.sync.dma_start(
                            cc_in[tt * 128:(tt + 1) * 128, :], part[:])

                    # ---------- phase F: launch ReduceScatter ----------
                    cc_out = dpool.tile([128, H], bf16, name=f"cc_out{b}",
                                        bufs=1)
                    nc.gpsimd.collective_compute(
                        "ReduceScatter", ALU.add,
                        replica_groups=[list(range(N_CORES))],
                        ins=[cc_in.opt()], outs=[cc_out.opt()])
                    cc_outs.append(cc_out)

                # ---------- deferred: residual + LN per batch ----------
                for b in range(B):
                    x_sb = lnpool.tile([128, H], bf16, name="x_sb", tag="lnbf", bufs=4)
                    x_dma = nc.sync.dma_start(x_sb[:], cc_outs[b][:])
                    add_dep_helper(x_dma.ins, last_part_dma.ins, sync=True,
                                   reason="defer LN past all compute")
                    qres_sb = lnpool.tile([128, H], f32, name="qres_sb", tag="lnbig", bufs=8)
                    nc.sync.dma_start(qres_sb[:], qres_d[b])
                    x2 = lnpool.tile([128, H], f32, name="x2", tag="lnbig", bufs=8)
                    xsum = lnpool.tile([128, 1], f32, name="xsum")
                    nc.vector.scalar_tensor_tensor(
                        x2[:], x_sb[:], 1.0, qres_sb[:],
                        op0=ALU.mult, op1=ALU.add, accum_out=xsum[:])
                    sqs = lnpool.tile([128, H], f32, name="sqs", tag="lnbig", bufs=8)
                    x2sum = lnpool.tile([128, 1], f32, name="x2sum")
                    nc.scalar.activation(sqs[:], x2[:], AF.Square,
                                         accum_out=x2sum[:])
                    negmu = lnpool.tile([128, 1], f32, name="negmu")
                    nc.vector.tensor_scalar(negmu[:], xsum[:], -1.0 / H, None,
                                            op0=ALU.mult)
                    varpe = lnpool.tile([128, 1], f32, name="varpe")
                    nc.vector.tensor_scalar(varpe[:], x2sum[:], 1.0 / H,
                                            LN_EPS, op0=ALU.mult, op1=ALU.add)
                    mu2 = lnpool.tile([128, 1], f32, name="mu2")
                    nc.vector.tensor_tensor(mu2[:], negmu[:], negmu[:],
                                            op=ALU.mult)
                    varr = lnpool.tile([128, 1], f32, name="varr")
                    nc.vector.tensor_tensor(varr[:], varpe[:], mu2[:],
                                            op=ALU.subtract)
                    rstd = rsqrt_newton(lnpool, varr, "rs")
                    xhat = lnpool.tile([128, H], f32, name="xhat", tag="lnbig", bufs=8)
                    nc.vector.tensor_scalar(xhat[:], x2[:], negmu[:], rstd[:],
                                            op0=ALU.add, op1=ALU.mult)
                    t1 = lnpool.tile([128, H], f32, name="t1", tag="lnbig", bufs=8)
                    nc.vector.tensor_tensor(t1[:], xhat[:], lng_bc[:],
                                            op=ALU.mult)
                    y_sb = lnpool.tile([128, H], f32, name="y_sb", tag="lnbig", bufs=8)
                    nc.vector.tensor_tensor(y_sb[:], t1[:], lnb_bc[:],
                                            op=ALU.add)
                    nc.sync.dma_start(y_d[b], y_sb[:])

    nc.compile()
    return nc


def _get_compiled():
    if "nc" not in _CACHE:
        _CACHE["nc"] = _build()
    return _CACHE["nc"]


def kernel(q, k, v, w_q, w_k, w_v, w_o, a_gate, k_gate, fc_w, fc_b,
           ln_gamma, ln_beta):
    import ml_dtypes
    from concourse.bass_utils import run_bass_kernel_spmd

    q = np.asarray(q, np.float32)
    k = np.asarray(k, np.float32)
    v = np.asarray(v, np.float32)
    w_q = np.asarray(w_q, np.float32)
    w_k = np.asarray(w_k, np.float32)
    w_v = np.asarray(w_v, np.float32)
    w_o = np.asarray(w_o, np.float32)
    a_gate = np.asarray(a_gate, np.float32)
    k_gate = np.asarray(k_gate, np.float32)
    fc_w = np.asarray(fc_w, np.float32)
    fc_b = np.asarray(fc_b, np.float32)
    ln_gamma = np.asarray(ln_gamma, np.float32)
    ln_beta = np.asarray(ln_beta, np.float32)

    nc = _get_compiled()

    qT = np.ascontiguousarray(q.reshape(B * T, H).T).astype(ml_dtypes.bfloat16)
    kT = np.ascontiguousarray(k.reshape(B * T, H).T).astype(ml_dtypes.bfloat16)
    vT = np.ascontiguousarray(v.reshape(B * T, H).T).astype(ml_dtypes.bfloat16)
    fcb = np.ascontiguousarray(fc_b.reshape(1, H)).astype(ml_dtypes.bfloat16)
    ag = np.ascontiguousarray(a_gate.reshape(1, NH))
    kg = np.ascontiguousarray(k_gate.reshape(1, NH))
    lng = np.ascontiguousarray(ln_gamma.reshape(1, H))
    lnb = np.ascontiguousarray(ln_beta.reshape(1, H))

    in_maps = []
    for n in range(N_CORES):
        sel = np.zeros((1, NH), np.float32)
        sel[0, n] = 1.0
        in_maps.append({
            "qT": qT, "kT": kT, "vT": vT,
            "wq": np.ascontiguousarray(w_q[n]).astype(ml_dtypes.bfloat16),
            "wk": np.ascontiguousarray(w_k[n]).astype(ml_dtypes.bfloat16),
            "wv": np.ascontiguousarray(w_v[n]).astype(ml_dtypes.bfloat16),
            "wo": np.ascontiguousarray(w_o[n]),
            "fcw": np.ascontiguousarray(fc_w).astype(ml_dtypes.bfloat16),
            "fcb": fcb,
            "ag": ag, "kg": kg, "sel": sel,
            "lng": lng, "lnb": lnb,
            "qres": np.ascontiguousarray(q[:, n * 128:(n + 1) * 128, :]),
        })

    res = run_bass_kernel_spmd(nc, in_maps, core_ids=list(range(N_CORES)))
    _CACHE["last_res"] = res

    y = np.empty((B, T, H), np.float32)
    attn = np.empty((NH * B, T, T), np.float32)
    for n in range(N_CORES):
        y[:, n * 128:(n + 1) * 128, :] = res.results[n]["y_out"]
        attn[n * B:(n + 1) * B] = res.results[n]["attn_out"]
    return y, attn


# revision 17
# speedup vs baseline: 1.3054x; 1.0418x over previous
"""BranchedAttention Trainium2 kernel (8-core head-parallel SPMD).

Strategy (head parallel per sharding hint):
  - core n owns head n: projections, attention, folded output projection.
  - w_o @ fc_w are back-to-back linear maps (selu comes after fc_w), so they
    are folded on device into W2[n] = softmax(k_gate)[n] * (w_o[n] @ fc_w).
  - scores computed twice (natural [q,s] for softmax+output, transposed [s,q]
    for the attn@v matmul) to avoid on-device fp32 transposes of attention.
    attn@v consumes unnormalized exp; the softmax denominator is folded in
    after the V matmul (linearity).
  - head-sum via ReduceScatter per batch; residual+layernorm on the shard.

Self-contained: hardcodes all shapes from the problem spec.
"""

import numpy as np

B = 4
T = 1024          # q_len == seq_len
H = 1024          # hidden
NH = 8            # heads
DK = 128          # per-head dim
E = 2 * H         # fc input dim
N_CORES = 8
LN_EPS = 1e-6
INV_SQRT_DK = 1.0 / float(np.sqrt(DK))
SELU_SCALE = 1.0507009873554805
SELU_ALPHA = 1.6732632423543772

_CACHE = {}


def _build():
    import concourse.bass as bass
    import concourse.mybir as mybir
    import concourse.tile as tile
    from concourse import bacc
    from concourse.masks import make_identity
    from concourse.tile_rust import add_dep_helper

    f32 = mybir.dt.float32
    f32r = mybir.dt.float32r
    bf16 = mybir.dt.bfloat16
    u32 = mybir.dt.uint32
    AF = mybir.ActivationFunctionType
    ALU = mybir.AluOpType

    nc = bacc.Bacc("TRN2", target_bir_lowering=False, debug=False,
                   num_devices=N_CORES)

    # ---- I/O ----
    qT_d = nc.dram_tensor("qT", [H, B * T], bf16, kind="ExternalInput")
    kT_d = nc.dram_tensor("kT", [H, B * T], bf16, kind="ExternalInput")
    vT_d = nc.dram_tensor("vT", [H, B * T], bf16, kind="ExternalInput")
    wq_d = nc.dram_tensor("wq", [H, DK], bf16, kind="ExternalInput")
    wk_d = nc.dram_tensor("wk", [H, DK], bf16, kind="ExternalInput")
    wv_d = nc.dram_tensor("wv", [H, DK], bf16, kind="ExternalInput")
    wo_d = nc.dram_tensor("wo", [DK, E], f32r, kind="ExternalInput")
    fcw_d = nc.dram_tensor("fcw", [E, H], bf16, kind="ExternalInput")
    fcb_d = nc.dram_tensor("fcb", [1, H], bf16, kind="ExternalInput")
    ag_d = nc.dram_tensor("ag", [1, NH], f32, kind="ExternalInput")
    kg_d = nc.dram_tensor("kg", [1, NH], f32, kind="ExternalInput")
    sel_d = nc.dram_tensor("sel", [1, NH], f32, kind="ExternalInput")
    lng_d = nc.dram_tensor("lng", [1, H], f32, kind="ExternalInput")
    lnb_d = nc.dram_tensor("lnb", [1, H], f32, kind="ExternalInput")
    qres_d = nc.dram_tensor("qres", [B, 128, H], f32, kind="ExternalInput")

    attn_d = nc.dram_tensor("attn_out", [B, T, T], f32, kind="ExternalOutput")
    y_d = nc.dram_tensor("y_out", [B, 128, H], f32, kind="ExternalOutput")

    def rsqrt_newton(pool, varr, name):
        """1/sqrt(varr) on [128,1] via quake initial guess + 3 Newton steps."""
        t0 = pool.tile([128, 1], u32, name=f"{name}_t0")
        nc.vector.tensor_scalar(t0[:], varr[:].bitcast(u32), 1, None,
                                op0=ALU.logical_shift_right)
        y0 = pool.tile([128, 1], f32, name=f"{name}_y0")
        nc.vector.tensor_tensor(y0[:].bitcast(u32), magic[:], t0[:],
                                op=ALU.subtract)
        ah = pool.tile([128, 1], f32, name=f"{name}_ah")
        nc.vector.tensor_scalar(ah[:], varr[:], 0.5, None, op0=ALU.mult)
        ycur = y0
        for it in range(3):
            sq = pool.tile([128, 1], f32, name=f"{name}_sq{it}")
            nc.vector.tensor_tensor(sq[:], ycur[:], ycur[:], op=ALU.mult)
            u = pool.tile([128, 1], f32, name=f"{name}_u{it}")
            nc.vector.tensor_tensor(u[:], sq[:], ah[:], op=ALU.mult)
            w2c = pool.tile([128, 1], f32, name=f"{name}_w2{it}")
            nc.vector.tensor_scalar(w2c[:], u[:], -1.0, 1.5,
                                    op0=ALU.mult, op1=ALU.add)
            yn = pool.tile([128, 1], f32, name=f"{name}_yn{it}")
            nc.vector.tensor_tensor(yn[:], ycur[:], w2c[:], op=ALU.mult)
            ycur = yn
        return ycur

    with tile.TileContext(nc) as tc:
        with (
            tc.tile_pool(name="const", bufs=1) as cpool,
            tc.tile_pool(name="wpool", bufs=1) as wpool,
        ):
            # ---- constants ----
            ident = cpool.tile([128, 128], f32)
            make_identity(nc, ident[:])
            magic = cpool.tile([128, 1], u32)
            nc.gpsimd.memset(magic[:], 0x5F3759DF)

            # ---- gate softmaxes; select this head's entries ----
            ag_t = cpool.tile([1, NH], f32)
            kg_t = cpool.tile([1, NH], f32)
            sel_t = cpool.tile([1, NH], f32)
            nc.sync.dma_start(ag_t[:], ag_d[:])
            nc.sync.dma_start(kg_t[:], kg_d[:])
            nc.sync.dma_start(sel_t[:], sel_d[:])

            def gate_scalar(gt, name):
                ge = cpool.tile([1, NH], f32, name=f"{name}_ge")
                gs = cpool.tile([1, 1], f32, name=f"{name}_gs")
                nc.scalar.activation(ge[:], gt[:], AF.Exp, accum_out=gs[:])
                gr = cpool.tile([1, 1], f32, name=f"{name}_gr")
                nc.vector.reciprocal(gr[:], gs[:])
                picked = cpool.tile([1, NH], f32, name=f"{name}_picked")
                nc.vector.tensor_tensor(picked[:], ge[:], sel_t[:], op=ALU.mult)
                psum_ = cpool.tile([1, 1], f32, name=f"{name}_psum")
                nc.vector.reduce_sum(psum_[:], picked[:],
                                     axis=mybir.AxisListType.X)
                out = cpool.tile([1, 1], f32, name=f"{name}_sm")
                nc.vector.tensor_tensor(out[:], psum_[:], gr[:], op=ALU.mult)
                return out

            sma = gate_scalar(ag_t, "a")     # softmax(a_gate)[n]
            smk = gate_scalar(kg_t, "k")     # softmax(k_gate)[n]
            sma_bc = cpool.tile([128, 1], f32)
            smk_bc = cpool.tile([128, 1], f32)
            nc.gpsimd.partition_broadcast(sma_bc[:], sma[:])
            nc.gpsimd.partition_broadcast(smk_bc[:], smk[:])
            c1 = cpool.tile([128, 1], f32)   # selu_scale * sm_a
            nc.vector.tensor_scalar(c1[:], sma_bc[:], SELU_SCALE, None,
                                    op0=ALU.mult)
            c2 = cpool.tile([128, 1], f32)   # selu_scale * alpha * sm_a
            nc.vector.tensor_scalar(c2[:], sma_bc[:], SELU_SCALE * SELU_ALPHA,
                                    None, op0=ALU.mult)
            lnc2 = cpool.tile([128, 1], f32)
            nc.scalar.activation(lnc2[:], c2[:], AF.Ln)

            # ---- LN gamma/beta broadcast ----
            lng_row = cpool.tile([1, H], f32)
            lnb_row = cpool.tile([1, H], f32)
            nc.sync.dma_start(lng_row[:], lng_d[:])
            nc.sync.dma_start(lnb_row[:], lnb_d[:])
            lng_bc = cpool.tile([128, H], f32)
            lnb_bc = cpool.tile([128, H], f32)
            nc.gpsimd.partition_broadcast(lng_bc[:], lng_row[:])
            nc.gpsimd.partition_broadcast(lnb_bc[:], lnb_row[:])

            # ---- per-head projection weights ----
            wq_sb = wpool.tile([128, H], bf16)   # col block h: wq[h*128:+128,:]
            wk_sb = wpool.tile([128, H], bf16)
            wv_sb = wpool.tile([128, H], bf16)
            for h in range(8):
                nc.sync.dma_start(wq_sb[:, h * 128:(h + 1) * 128],
                                  wq_d[h * 128:(h + 1) * 128, :])
                nc.sync.dma_start(wk_sb[:, h * 128:(h + 1) * 128],
                                  wk_d[h * 128:(h + 1) * 128, :])
                nc.sync.dma_start(wv_sb[:, h * 128:(h + 1) * 128],
                                  wv_d[h * 128:(h + 1) * 128, :])

            fcb_sb = cpool.tile([1, H], bf16)
            nc.sync.dma_start(fcb_sb[:], fcb_d[:])

            # ---- fold W2 = smk * (w_o @ fc_w)  [DK, H] ----
            W2_sb = wpool.tile([128, H], bf16)
            with (
                tc.tile_pool(name="stage", bufs=1) as stage,
                tc.tile_pool(name="stage_ps", bufs=2, space="PSUM") as stage_ps,
            ):
                wo_sb = stage.tile([128, E], f32r)
                nc.sync.dma_start(wo_sb[:], wo_d[:])
                # scale by softmax(k_gate)[n]
                nc.vector.tensor_scalar(wo_sb[:].bitcast(f32),
                                        wo_sb[:].bitcast(f32), smk_bc[:],
                                        None, op0=ALU.mult)
                # transpose wo -> woT [E, DK] (16 col blocks of [128,128])
                woT_sb = stage.tile([128, 16 * 128], bf16)
                for g in range(4):
                    tr_ps = stage_ps.tile([128, 512], f32, name="tr_ps")
                    for j in range(4):
                        e = g * 4 + j
                        nc.tensor.transpose(
                            tr_ps[:, j * 128:(j + 1) * 128],
                            wo_sb[:, e * 128:(e + 1) * 128].bitcast(f32),
                            ident[:])
                    nc.vector.tensor_copy(
                        woT_sb[:, g * 512:(g + 1) * 512], tr_ps[:])
                # fcw chunks + accumulate W2
                w2_halves = []
                for half in range(2):
                    w2_ps_h = stage_ps.tile([128, 512], f32,
                                            name=f"w2_ps{half}", bufs=1)
                    w2_halves.append(w2_ps_h)
                for e in range(16):
                    fcw_t = stage.tile([128, H], bf16, name="fcw_t", bufs=3)
                    nc.sync.dma_start(fcw_t[:],
                                      fcw_d[e * 128:(e + 1) * 128, :])
                    for half in range(2):
                        nc.tensor.matmul(
                            w2_halves[half][:],
                            woT_sb[:, e * 128:(e + 1) * 128],
                            fcw_t[:, half * 512:(half + 1) * 512],
                            start=(e == 0), stop=(e == 15))
                for half in range(2):
                    nc.vector.tensor_copy(
                        W2_sb[:, half * 512:(half + 1) * 512],
                        w2_halves[half][:])

            # ---- main pools ----
            with (
                tc.tile_pool(name="stream", bufs=6) as stream,
                tc.tile_pool(name="proj", bufs=2) as proj,
                tc.tile_pool(name="apool", bufs=2) as apool,
                tc.tile_pool(name="fpool", bufs=2) as fpool,
                tc.tile_pool(name="lnpool", bufs=2) as lnpool,
                tc.tile_pool(name="mps", bufs=1, space="PSUM") as mps,
                tc.tile_pool(name="dpool", bufs=2, space="DRAM") as dpool,
            ):
                cc_outs = []
                for b in range(B):
                    c0 = b * T  # column offset of this batch in [H, B*T]

                    # ---------- phase A: projections ----------
                    qh_ps = mps.tile([128, T], f32, name="qh_ps", tag="acc1")
                    kh_ps = mps.tile([128, T], f32, name="kh_ps", tag="acc2")
                    for h in range(8):
                        qch = stream.tile([128, T], bf16, name="qch")
                        nc.sync.dma_start(
                            qch[:], qT_d[h * 128:(h + 1) * 128, c0:c0 + T])
                        for hf in range(2):
                            nc.tensor.matmul(
                                qh_ps[:, hf * 512:(hf + 1) * 512],
                                wq_sb[:, h * 128:(h + 1) * 128],
                                qch[:, hf * 512:(hf + 1) * 512],
                                start=(h == 0), stop=(h == 7))
                        kch = stream.tile([128, T], bf16, name="kch")
                        nc.sync.dma_start(
                            kch[:], kT_d[h * 128:(h + 1) * 128, c0:c0 + T])
                        for hf in range(2):
                            nc.tensor.matmul(
                                kh_ps[:, hf * 512:(hf + 1) * 512],
                                wk_sb[:, h * 128:(h + 1) * 128],
                                kch[:, hf * 512:(hf + 1) * 512],
                                start=(h == 0), stop=(h == 7))
                    qhT_sb = proj.tile([128, T], f32r, name="qhT_sb")
                    khT_sb = proj.tile([128, T], f32r, name="khT_sb")
                    nc.vector.tensor_copy(qhT_sb[:], qh_ps[:])
                    nc.scalar.copy(khT_sb[:], kh_ps[:])
                    vh_ps = mps.tile([128, T], f32, name="vh_ps", tag="acc1")
                    for h in range(8):
                        vch = stream.tile([128, T], bf16, name="vch")
                        nc.sync.dma_start(
                            vch[:], vT_d[h * 128:(h + 1) * 128, c0:c0 + T])
                        for hf in range(2):
                            nc.tensor.matmul(
                                vh_ps[:, hf * 512:(hf + 1) * 512],
                                wv_sb[:, h * 128:(h + 1) * 128],
                                vch[:, hf * 512:(hf + 1) * 512],
                                start=(h == 0), stop=(h == 7))
                    # vhT [d, s] -> transpose to vh natural [s, d] col blocks
                    vhT_sb = proj.tile([128, T], f32, name="vhT_sb")
                    nc.vector.tensor_copy(vhT_sb[:], vh_ps[:])
                    vh_sb = proj.tile([128, T], bf16, name="vh_sb")
                    for g in range(2):
                        tr_ps = mps.tile([128, 512], f32, name="tr_ps",
                                         tag=f"rot{g}")
                        for j in range(4):
                            st = g * 4 + j
                            nc.tensor.transpose(
                                tr_ps[:, j * 128:(j + 1) * 128],
                                vhT_sb[:, st * 128:(st + 1) * 128],
                                ident[:])
                        nc.vector.tensor_copy(
                            vh_sb[:, g * 512:(g + 1) * 512], tr_ps[:])

                    # ---------- phase B: scores + softmax (natural) ----------
                    sumsT_sb = apool.tile([1, T], bf16, name="sumsT_sb")
                    recips = []
                    for qt in range(8):
                        sc_ps = mps.tile([128, T], f32, name="sc_ps",
                                         tag=f"rot{qt % 2}")
                        for hf in range(2):
                            nc.tensor.matmul(
                                sc_ps[:, hf * 512:(hf + 1) * 512],
                                qhT_sb[:, qt * 128:(qt + 1) * 128],
                                khT_sb[:, hf * 512:(hf + 1) * 512],
                                start=True, stop=True)
                        exp_sb = apool.tile([128, T], f32, name="exp_sb", bufs=3)
                        sums = apool.tile([128, 1], f32, name="sums")
                        nc.scalar.activation(exp_sb[:], sc_ps[:], AF.Exp,
                                             scale=INV_SQRT_DK,
                                             accum_out=sums[:])
                        recip = apool.tile([128, 1], f32, name="recip",
                                           bufs=12)
                        recips.append(recip)
                        nc.vector.reciprocal(recip[:], sums[:])
                        nc.vector.tensor_scalar(exp_sb[:], exp_sb[:],
                                                recip[:], None, op0=ALU.mult)
                        nc.sync.dma_start(
                            attn_d[b, qt * 128:(qt + 1) * 128, :], exp_sb[:])
                        # sums -> transposed [1,128] into sumsT_sb (bias row)
                        rtp = mps.tile([1, 128], f32, name="rtp", tag="acc2")
                        nc.tensor.transpose(rtp[:], sums[:], ident[:])
                        nc.vector.tensor_copy(
                            sumsT_sb[0:1, qt * 128:(qt + 1) * 128], rtp[:])

                    # ---------- phase C+D: scoresT + exp, attn @ v ----------
                    outT_ps = mps.tile([128, T], f32, name="outT_ps",
                                       tag="acc1")
                    for st in range(8):
                        scT_ps = mps.tile([128, T], f32, name="scT_ps",
                                          tag=f"rot{st % 2}")
                        for hf in range(2):
                            nc.tensor.matmul(
                                scT_ps[:, hf * 512:(hf + 1) * 512],
                                khT_sb[:, st * 128:(st + 1) * 128],
                                qhT_sb[:, hf * 512:(hf + 1) * 512],
                                start=True, stop=True)
                        expT = apool.tile([128, T], bf16, name="expT", bufs=3)
                        nc.scalar.activation(expT[:], scT_ps[:],
                                             AF.Exp, scale=INV_SQRT_DK)
                        for hf in range(2):
                            nc.tensor.matmul(
                                outT_ps[:, hf * 512:(hf + 1) * 512],
                                vh_sb[:, st * 128:(st + 1) * 128],
                                expT[:, hf * 512:(hf + 1) * 512],
                                start=(st == 0), stop=(st == 7))
                    outT_sb = proj.tile([128, T], bf16, name="outT_sb")
                    nc.vector.tensor_copy(outT_sb[:], outT_ps[:])

                    # ---------- phase E: f = outT.T @ W2 + fcb; selu; gate ---
                    cc_in = dpool.tile([T, H], bf16, name="cc_in")
                    for tt in range(8):
                        f_ps = mps.tile([128, H], f32, name="f_ps",
                                        tag=f"rot{tt % 2}")
                        for hf in range(2):
                            nc.tensor.matmul(
                                f_ps[:, hf * 512:(hf + 1) * 512],
                                outT_sb[:, tt * 128:(tt + 1) * 128],
                                W2_sb[:, hf * 512:(hf + 1) * 512],
                                start=True, stop=False)
                            # bias row: += sums[t] * fcb[h]
                            nc.tensor.matmul(
                                f_ps[:, hf * 512:(hf + 1) * 512],
                                sumsT_sb[0:1, tt * 128:(tt + 1) * 128],
                                fcb_sb[0:1, hf * 512:(hf + 1) * 512],
                                start=False, stop=True)
                        # normalized pre-selu F = recip[t] * f_ps
                        rc = recips[tt]
                        c1r = fpool.tile([128, 1], f32, name="c1r")
                        nc.vector.tensor_tensor(c1r[:], c1[:], rc[:],
                                                op=ALU.mult)
                        e2 = fpool.tile([128, H], f32, name="e2")
                        nc.scalar.activation(e2[:], f_ps[:], AF.Exp,
                                             bias=lnc2[:], scale=rc[:])
                        rr = fpool.tile([128, H], f32, name="rr")
                        nc.vector.tensor_scalar(rr[:], f_ps[:], 0.0, c1r[:],
                                                op0=ALU.max, op1=ALU.mult)
                        part = fpool.tile([128, H], bf16, name="part")
                        nc.vector.scalar_tensor_tensor(
                            part[:], e2[:], c2[:], rr[:],
                            op0=ALU.min, op1=ALU.add)
                        last_part_dma = nc.sync.dma_start(
                            cc_in[tt * 128:(tt + 1) * 128, :], part[:])

                    # ---------- phase F: launch ReduceScatter ----------
                    cc_out = dpool.tile([128, H], bf16, name=f"cc_out{b}",
                                        bufs=1)
                    for ch in range(2):
                        nc.gpsimd.collective_compute(
                            "ReduceScatter", ALU.add,
                            replica_groups=[list(range(N_CORES))],
                            ins=[cc_in[ch * 512:(ch + 1) * 512, :].opt()],
                            outs=[cc_out[ch * 64:(ch + 1) * 64, :].opt()])
                    cc_outs.append(cc_out)

                # ---------- deferred: residual + LN per batch ----------
                for b in range(B):
                    x_sb = lnpool.tile([128, H], bf16, name="x_sb", tag="lnbf", bufs=4)
                    x_dma = nc.sync.dma_start(x_sb[:], cc_outs[b][:])
                    add_dep_helper(x_dma.ins, last_part_dma.ins, sync=True,
                                   reason="defer LN past all compute")
                    qres_sb = lnpool.tile([128, H], f32, name="qres_sb", tag="lnbig", bufs=8)
                    nc.sync.dma_start(qres_sb[:], qres_d[b])
                    x2 = lnpool.tile([128, H], f32, name="x2", tag="lnbig", bufs=8)
                    xsum = lnpool.tile([128, 1], f32, name="xsum")
                    nc.vector.scalar_tensor_tensor(
                        x2[:], x_sb[:], 1.0, qres_sb[:],
                        op0=ALU.mult, op1=ALU.add, accum_out=xsum[:])
                    sqs = lnpool.tile([128, H], f32, name="sqs", tag="lnbig", bufs=8)
                    x2sum = lnpool.tile([128, 1], f32, name="x2sum")
                    nc.scalar.activation(sqs[:], x2[:], AF.Square,
                                         accum_out=x2sum[:])
                    negmu = lnpool.tile([128, 1], f32, name="negmu")
                    nc.vector.tensor_scalar(negmu[:], xsum[:], -1.0 / H, None,
                                            op0=ALU.mult)
                    varpe = lnpool.tile([128, 1], f32, name="varpe")
                    nc.vector.tensor_scalar(varpe[:], x2sum[:], 1.0 / H,
                                            LN_EPS, op0=ALU.mult, op1=ALU.add)
                    mu2 = lnpool.tile([128, 1], f32, name="mu2")
                    nc.vector.tensor_tensor(mu2[:], negmu[:], negmu[:],
                                            op=ALU.mult)
                    varr = lnpool.tile([128, 1], f32, name="varr")
                    nc.vector.tensor_tensor(varr[:], varpe[:], mu2[:],
                                            op=ALU.subtract)
                    rstd = rsqrt_newton(lnpool, varr, "rs")
                    xhat = lnpool.tile([128, H], f32, name="xhat", tag="lnbig", bufs=8)
                    nc.vector.tensor_scalar(xhat[:], x2[:], negmu[:], rstd[:],
                                            op0=ALU.add, op1=ALU.mult)
                    t1 = lnpool.tile([128, H], f32, name="t1", tag="lnbig", bufs=8)
                    nc.vector.tensor_tensor(t1[:], xhat[:], lng_bc[:],
                                            op=ALU.mult)
                    y_sb = lnpool.tile([128, H], f32, name="y_sb", tag="lnbig", bufs=8)
                    nc.vector.tensor_tensor(y_sb[:], t1[:], lnb_bc[:],
                                            op=ALU.add)
                    nc.sync.dma_start(y_d[b], y_sb[:])

    nc.compile()
    return nc


def _get_compiled():
    if "nc" not in _CACHE:
        _CACHE["nc"] = _build()
    return _CACHE["nc"]


def kernel(q, k, v, w_q, w_k, w_v, w_o, a_gate, k_gate, fc_w, fc_b,
           ln_gamma, ln_beta):
    import ml_dtypes
    from concourse.bass_utils import run_bass_kernel_spmd

    q = np.asarray(q, np.float32)
    k = np.asarray(k, np.float32)
    v = np.asarray(v, np.float32)
    w_q = np.asarray(w_q, np.float32)
    w_k = np.asarray(w_k, np.float32)
    w_v = np.asarray(w_v, np.float32)
    w_o = np.asarray(w_o, np.float32)
    a_gate = np.asarray(a_gate, np.float32)
    k_gate = np.asarray(k_gate, np.float32)
    fc_w = np.asarray(fc_w, np.float32)
    fc_b = np.asarray(fc_b, np.float32)
    ln_gamma = np.asarray(ln_gamma, np.float32)
    ln_beta = np.asarray(ln_beta, np.float32)

    nc = _get_compiled()

    qT = np.ascontiguousarray(q.reshape(B * T, H).T).astype(ml_dtypes.bfloat16)
    kT = np.ascontiguousarray(k.reshape(B * T, H).T).astype(ml_dtypes.bfloat16)
    vT = np.ascontiguousarray(v.reshape(B * T, H).T).astype(ml_dtypes.bfloat16)
    fcb = np.ascontiguousarray(fc_b.reshape(1, H)).astype(ml_dtypes.bfloat16)
    ag = np.ascontiguousarray(a_gate.reshape(1, NH))
    kg = np.ascontiguousarray(k_gate.reshape(1, NH))
    lng = np.ascontiguousarray(ln_gamma.reshape(1, H))
    lnb = np.ascontiguousarray(ln_beta.reshape(1, H))

    in_maps = []
    for n in range(N_CORES):
        sel = np.zeros((1, NH), np.float32)
        sel[0, n] = 1.0
        in_maps.append({
            "qT": qT, "kT": kT, "vT": vT,
            "wq": np.ascontiguousarray(w_q[n]).astype(ml_dtypes.bfloat16),
            "wk": np.ascontiguousarray(w_k[n]).astype(ml_dtypes.bfloat16),
            "wv": np.ascontiguousarray(w_v[n]).astype(ml_dtypes.bfloat16),
            "wo": np.ascontiguousarray(w_o[n]),
            "fcw": np.ascontiguousarray(fc_w).astype(ml_dtypes.bfloat16),
            "fcb": fcb,
            "ag": ag, "kg": kg, "sel": sel,
            "lng": lng, "lnb": lnb,
            "qres": np.ascontiguousarray(np.concatenate(
                [q[:, n * 64:(n + 1) * 64, :],
                 q[:, 512 + n * 64:512 + (n + 1) * 64, :]], axis=1)),
        })

    res = run_bass_kernel_spmd(nc, in_maps, core_ids=list(range(N_CORES)))
    _CACHE["last_res"] = res

    y = np.empty((B, T, H), np.float32)
    attn = np.empty((NH * B, T, T), np.float32)
    for n in range(N_CORES):
        yo = res.results[n]["y_out"]
        y[:, n * 64:(n + 1) * 64, :] = yo[:, 0:64, :]
        y[:, 512 + n * 64:512 + (n + 1) * 64, :] = yo[:, 64:128, :]
        attn[n * B:(n + 1) * B] = res.results[n]["attn_out"]
    return y, attn
